# revision 1
# baseline (speedup 1.0000x reference)
"""Trainium2 Bass kernel for nn_EwaldPotential — v2 (fp32r rework).

Math per graph (all-real form of the complex reference):
  p[i,k] = (r[i,:]/box) @ kvecs[k,:]      (phase in cycles)
  C = cos(2*pi*p), S = sin(2*pi*p)
  KRT[d,k] = sum_i k[i,d] C[i,k]; KIT from S; VRT/VIT likewise from v
  A = q @ KRT, B = q @ KIT;  L = C*A - S*B
  P = softmax_k(L);  out = (P*C) @ VR - (P*S) @ VI

v2 design:
 - kpot + logits matmuls in float32r: 1 cycle/row (vs 4 for float32) at
   >=256-wide outputs. fp32r operands must come from instructions with
   float32r output dtype: kv/q/KRT/KIT and a rotating copy of C/S are
   produced by cheap on-chip casts (DVE tensor_scalar runs at the 2x
   DVE mode even for fp32).
 - ONE Sin per chunk: the S-path and C-path phase matmuls write one
   2-bank PSUM tile ([128,1024] fp32 = banks for -frac(p) and
   -frac(p+1/4)); a single ACT Sin with a [p,2,297] access pattern
   produces S and C side by side (both are sin(-2pi * frac)).
 - C/S kept in full fp32 for the logits/softmax elementwise chain
   (precision), f32r only where the PE consumes them.
 - softmax fused: tensor_tensor_reduce gives -L and -max(L) in one DVE
   op; exp(+accum) gives the row sum; G=(E*rrs)*C and H=(E*rrs)*S are
   single fused scalar_tensor_tensor ops (no separate P tensor).
 - engine balance: DVE: T1, TTR, recip, G.  Pool: T2 (as STT: the
   GPSIMD cost table rates TensorScalarTensor 0.6 vs 0.42 for plain
   multiplies), H, og copies.  ACT: exp, GHT copy.  PE: all matmuls +
   transposes.
 - all K-wide ops run on the 297 valid columns; G/H transpose pads are
   zeroed once (static ring buffers).
 - input DMAs merged (lhs/rhs phase operands share one 2-byte tensor)
   and ordered so compute starts as soon as possible.
"""
import json
import numpy as np
import ml_dtypes

import concourse.bass as bass
import concourse.tile as tile
from concourse import mybir
from concourse.bass_utils import run_bass_kernel_spmd

# ---------------------------------------------------------------- constants
B = 8
N_NODE = 2048
D = 64
DL = 4.0
TWOPI = 2.0 * np.pi
K_SQ_MAX = (TWOPI / DL) ** 2
MAGIC = 12582912.0          # 1.5 * 2^23: fp32 add rounds to integer
KV = 297                    # valid k-point count
KVE = 298                   # even width for fp32r matmul outputs
NCHUNK = N_NODE // 128

FP32 = mybir.dt.float32
FP32R = mybir.dt.float32r
BF16 = mybir.dt.bfloat16
FP16 = mybir.dt.float16
U16 = mybir.dt.uint16

# ------------------------------------------------- walrus wait-split patch
_ws_counter = [0]


def _split_waits_json(bir_bytes: bytes) -> bytes:
    d = json.loads(bir_bytes)
    changed = False
    for fn in d.get("functions", []):
        for blk in fn.get("blocks", []):
            out = []
            for ins in blk.get("instructions", []):
                si = ins.get("sync_info")
                ow = (si or {}).get("on_wait") or []
                if len(ow) > 1:
                    changed = True
                    for w in ow[:-1]:
                        _ws_counter[0] += 1
                        out.append({
                            "debug": ins.get("debug", 0),
                            "engine": ins.get("engine"),
                            "ins": [], "outs": [],
                            "name": f"I-wsplit{_ws_counter[0]}",
                            "opcode": "NoOp",
                            "sync_info": {"on_wait": [w], "on_update": []},
                        })
                    si["on_wait"] = [ow[-1]]
                out.append(ins)
            blk["instructions"] = out
    return json.dumps(d).encode() if changed else bir_bytes


def _install_bir_patch():
    import concourse.bass_utils as bu
    import concourse.bass2jax as b2j

    if getattr(b2j.compile_bir_kernel, "_wait_split", False):
        return
    orig = bu.compile_bir_kernel

    def wrapped(bir_json, tmpdir, neff_name="file.neff"):
        return orig(_split_waits_json(bir_json), tmpdir, neff_name)

    wrapped._wait_split = True
    b2j.compile_bir_kernel = wrapped
    bu.compile_bir_kernel = wrapped


# ------------------------------------------------------------ device kernel
def _build_nc(cfg=None):
    cfg = {**{'ght_dve_cols': 0, 'work_bufs': 4,
              'ab_bufs': 2, 'ght_bufs': 2, 'gh_bufs': 4, 'scr_bufs': 3,
              'ph_bufs': 3, 'shift': 2, 'pair_ab': True, 'pair_gh': True,
              'scr': 'cast', 'debug': False}, **(cfg or {})}
    nc = bass.Bass("TRN2")
    # lhs cols 0:N = bf16 phase weights (rows 0:48), cols N:2N = fp16
    # residual weights (rows 0:35) — column-merged so fp16 rows stay at
    # partition offsets 0:3 / 32:35 (PE tile positions need 32-alignment).
    lhs = nc.dram_tensor("lhs", [48, 2 * N_NODE], U16, kind="ExternalInput")
    rhs = nc.dram_tensor("rhs", [48, 2 * KV], U16, kind="ExternalInput")
    kvcat = nc.dram_tensor("kvcat", [128, N_NODE], FP32, kind="ExternalInput")
    qt = nc.dram_tensor("qt", [64, N_NODE], FP32, kind="ExternalInput")
    # raw og layout: out_g[p, g*256 + b*64 + d] = out[g*512 + b*128 + p, d]
    # (host reassembles; keeps the output DMAs fully contiguous)
    out_g = nc.dram_tensor("out_g", [128, 1024], FP32, kind="ExternalOutput")
    dbg = {}
    if cfg['debug']:
        for nm, shape, dt in (("d_SC0", [128, 608], FP32),
                              ("d_KRT", [64, KVE], FP32),
                              ("d_KIT", [64, KVE], FP32),
                              ("d_T12", [128, 608], FP32),
                              ("d_nL", [128, KV], FP32),
                              ("d_sml", [128, 3], FP32),
                              ("d_E", [128, KV], FP32),
                              ("d_G", [128, 384], FP16),
                              ("d_H", [128, 384], FP16),
                              ("d_GHT", [128, 768], FP16),
                              ("d_VRsb", [128, 192], FP16),
                              ("d_VIsb", [128, 192], FP16),
                              ("d_og", [128, 512], FP32)):
            dbg[nm] = nc.dram_tensor(nm, shape, dt, kind="ExternalOutput")

    Sin = mybir.ActivationFunctionType.Sin
    Exp = mybir.ActivationFunctionType.Exp
    Copy = mybir.ActivationFunctionType.Copy
    mult = mybir.AluOpType.mult
    sub = mybir.AluOpType.subtract
    amin = mybir.AluOpType.min

    GB = cfg['gh_bufs']
    NH = N_NODE // 2

    with tile.TileContext(nc) as tc:
        with tc.tile_pool(name="consts", bufs=1) as consts, \
             tc.tile_pool(name="work", bufs=cfg['work_bufs']) as work, \
             tc.tile_pool(name="scr", bufs=cfg['scr_bufs']) as scr, \
             tc.tile_pool(name="ghts", bufs=cfg['ght_bufs']) as ghts, \
             tc.tile_pool(name="small", bufs=4) as small:

            t_lhs = consts.tile([48, 2 * N_NODE], U16, tag="lhs")
            t_rhs = consts.tile([48, 2 * KV], U16, tag="rhs")
            # kvcat rows: 0:64 = v, 64:128 = k (v_pot then lands on
            # partitions 0:64 for offset-0 PE transposes; q/k_pot live on
            # 64:128 and the logits matmuls use tile_position=(64,0)).
            t_kv = consts.tile([128, N_NODE], FP32, tag="kv")
            t_q128 = consts.tile([128, N_NODE], FP32, tag="q128")
            # order: phase weights first (phase1 starts ASAP), then kv, q
            nc.sync.dma_start(t_lhs[:, 0:N_NODE], lhs[:, 0:N_NODE])
            nc.sync.dma_start(t_rhs, rhs[:])
            nc.sync.dma_start(t_lhs[:, N_NODE:], lhs[:, N_NODE:])
            nc.sync.dma_start(t_kv[:, 0:NH], kvcat[:, 0:NH])
            nc.sync.dma_start(t_kv[:, NH:N_NODE], kvcat[:, NH:N_NODE])
            nc.sync.dma_start(t_q128[64:128, :], qt[:])

            lb = t_lhs[0:48, 0:N_NODE].bitcast(BF16)
            lh = t_lhs[0:35, N_NODE:2 * N_NODE].bitcast(FP16)
            rb = t_rhs[0:48, 0:KV].bitcast(BF16)
            rh = t_rhs[0:35, KV:2 * KV].bitcast(FP16)

            # kpot and logits both run in full 4-pass fp32: softmax is too
            # sensitive to fp32r's ~1e-4 operand rounding anywhere in the
            # logits chain (measured 3.8e-2 max rel vs the 2e-2 gate).
            kvrr = t_kv[:].rearrange("p (c d) -> p c d", c=NCHUNK)

            # S|C slabs per chunk: S at [0:304], C at [304:608]
            if cfg['scr'] == 'direct':
                t_SC = [None] * NCHUNK
            else:
                t_SC = [consts.tile([128, 608], FP32, tag=f"SC{c}",
                                    name=f"SCt{c}") for c in range(NCHUNK)]
                for c in range(NCHUNK):
                    # pad cols [297:304] / [601:608]: Sin never writes them
                    # but the full-width SCr cast (and kpot col 297) read them
                    pad = t_SC[c][:].rearrange(
                        "p (b x) -> p b x", b=2)[:, :, KV:304]
                    nc.vector.memset(pad, 0.0)

            from concourse.masks import make_identity
            t_ident = consts.tile([128, 128], FP16, tag="ident")
            make_identity(nc, t_ident)

            # G/H buffers: static rings of [128, 384] fp16; pad columns
            # [297:384] zeroed once so the j2 transposes read zeros and all
            # transposes run full-width (no partially-written PSUM)
            t_G = [consts.tile([128, 384], FP16, tag=f"G{i}", name=f"Gt{i}")
                   for i in range(GB)]
            t_H = [consts.tile([128, 384], FP16, tag=f"H{i}", name=f"Ht{i}")
                   for i in range(GB)]
            for i in range(GB):
                nc.vector.memset(t_G[i][:, KV:384], 0.0)
                nc.vector.memset(t_H[i][:, KV:384], 0.0)

            # PE warmup: burn the p-state ramp on dummy transposes so the
            # first real matmuls already run at a higher clock.
            with tc.tile_pool(name="ps_warm", bufs=1, space="PSUM") as ps_w:
                pw = ps_w.tile([128, 128], FP16, tag="warm")
                for _ in range(14):
                    nc.tensor.transpose(pw, t_ident, t_ident)

            # ---------------- phase 1: trig + k_pot/v_pot accumulation
            # PE program order is software-pipelined: kpot(c-1) is emitted
            # after the phase matmuls of chunk c so it never head-of-line
            # blocks on the Sin/cast chain.
            SCr = [None] * NCHUNK
            pend = []

            def emit_kpot(c):
                nc.tensor.matmul(psKC, kvrr[:, c, :],
                                 t_SC[c][:, 304:304 + KVE],
                                 start=(c == 0), stop=(c == NCHUNK - 1))
                nc.tensor.matmul(psKS, kvrr[:, c, :], t_SC[c][:, 0:KVE],
                                 start=(c == 0), stop=(c == NCHUNK - 1))

            with tc.tile_pool(name="ps_ph", bufs=cfg['ph_bufs'],
                              space="PSUM") as ps_ph, \
                 tc.tile_pool(name="ps_kc", bufs=1, space="PSUM") as ps_kc:
                psKC = ps_kc.tile([128, KVE], FP32, tag="KC")
                psKS = ps_kc.tile([128, KVE], FP32, tag="KS")
                for c in range(NCHUNK):
                    sl = slice(c * 128, (c + 1) * 128)
                    # 2-bank psum: -frac(p) at [0:297], -frac(p+1/4) at
                    # [512:809]; one Sin covers both via a [p,2,297] AP.
                    pSC = ps_ph.tile([128, 1024], FP32, tag="pSC")
                    nc.tensor.matmul(pSC[:, 0:KV], lb[0:8, sl], rb[0:8, :],
                                     start=True, stop=False, tile_position=(0, 0))
                    nc.tensor.matmul(pSC[:, 512:512 + KV], lb[32:42, sl],
                                     rb[32:42, :], start=True, stop=False,
                                     tile_position=(32, 0))
                    nc.tensor.matmul(pSC[:, 0:KV], lh[0:3, sl], rh[0:3, :],
                                     start=False, stop=True, tile_position=(0, 0))
                    nc.tensor.matmul(pSC[:, 512:512 + KV], lh[32:35, sl],
                                     rh[32:35, :], start=False, stop=True,
                                     tile_position=(32, 0))
                    if c >= 1:
                        emit_kpot(c - 1)
                    pin = pSC[:].rearrange("p (b x) -> p b x", b=2)[:, :, 0:KV]
                    if cfg['scr'] == 'direct':
                        # Sin writes fp32r directly; elementwise consumers
                        # read it via bitcast (costs ~1e-4 rounding on C/S).
                        SCr[c] = consts.tile([128, 608], FP32R,
                                             tag=f"SCd{c}", name=f"SCd{c}")
                        t_SC[c] = SCr[c]
                        sout = SCr[c][:].rearrange(
                            "p (b x) -> p b x", b=2)[:, :, 0:KV]
                        nc.scalar.activation(sout, pin, Sin, scale=-TWOPI)
                    else:
                        sout = t_SC[c][:].rearrange(
                            "p (b x) -> p b x", b=2)[:, :, 0:KV]
                        nc.scalar.activation(sout, pin, Sin, scale=-TWOPI)
                emit_kpot(NCHUNK - 1)

                # k_pot rows 64:128 -> full-fp32 rhs for the fp32 logits
                # matmuls (GPSIMD cannot touch PSUM, so both copies go DVE)
                t_KRT = consts.tile([128, KVE], FP32, tag="KRT")
                t_KIT = consts.tile([128, KVE], FP32, tag="KIT")
                nc.vector.tensor_copy(t_KRT[64:128, :], psKC[64:128, :])
                nc.vector.tensor_copy(t_KIT[64:128, :], psKS[64:128, :])
                # v_pot rows 0:64 -> fp16, zero-padded to 304 wide
                t_VRT = consts.tile([64, 384], FP16, tag="VRT")
                t_VIn = consts.tile([64, 384], FP16, tag="VIn")
                nc.vector.memset(t_VRT[:, KV:384], 0.0)
                nc.vector.memset(t_VIn[:, KV:384], 0.0)
                nc.scalar.activation(t_VRT[:, 0:KV], psKC[0:64, 0:KV], Copy)
                nc.scalar.activation(t_VIn[:, 0:KV], psKS[0:64, 0:KV], Copy,
                                     scale=-1.0)

            t_VRsb = consts.tile([128, 192], FP16, tag="VRsb")
            t_VIsb = consts.tile([128, 192], FP16, tag="VIsb")

            ps_ab = tc.alloc_tile_pool(name="ps_ab", bufs=cfg['ab_bufs'],
                                       space="PSUM")
            ps_ght = tc.alloc_tile_pool(name="ps_ght", bufs=cfg['ght_bufs'],
                                        space="PSUM")
            ps_out = tc.alloc_tile_pool(name="ps_out", bufs=2, space="PSUM")


            # ---------------- phase 2: logits, softmax, inverse transform
            # PE order is shifted: transposes+finals of chunk c-shift are
            # emitted after the logits matmuls of chunk c, so the PE never
            # waits on the softmax chain of the chunk it just fed.
            # Engine placement under the no-PSUM-on-GPSIMD rule:
            #   DVE: T1|T2 (one paired op), TTR, recip, og copy, GHT head
            #   Pool: G|H (one paired op, all-SBUF)
            #   ACT: exp, GHT tail
            SH = cfg['shift']
            psO = [None] * 2
            t_og_of = [None] * 2
            DC = cfg['ght_dve_cols']

            def emit_ab(c):
                sl = slice(c * 128, (c + 1) * 128)
                if c % 8 == 0:
                    psO[c // 8] = ps_out.tile([128, 512], FP32, tag="O",
                                              name=f"Ot{c//8}")
                # paired 2-bank psum: B at [0:KV], A at [512:512+KV] so one
                # DVE op multiplies [S|C] against [B|A] in a single pass.
                psAB = ps_ab.tile([128, 1024], FP32, tag="AB", name=f"psAB{c}")
                nc.tensor.matmul(psAB[:, 0:KVE], t_q128[64:128, sl],
                                 t_KIT[64:128, :], start=True, stop=True,
                                 tile_position=(64, 0))
                nc.tensor.matmul(psAB[:, 512:512 + KVE], t_q128[64:128, sl],
                                 t_KRT[64:128, :], start=True, stop=True,
                                 tile_position=(64, 0))
                return psAB

            rrs_of = [None] * NCHUNK

            def emit_softmax(c, psAB):
                Cc = t_SC[c][:, 304:304 + KV].bitcast(FP32)
                Sc = t_SC[c][:, 0:KV].bitcast(FP32)
                scp = t_SC[c][:].bitcast(FP32).rearrange(
                    "p (b x) -> p b x", b=2)[:, :, 0:KV]
                abp = psAB[:].rearrange("p (b x) -> p b x", b=2)[:, :, 0:KV]
                # T12 = [T2 | T1] = [S*B | C*A] in one DVE pass
                T12 = work.tile([128, 608], FP32, tag="T12", name=f"T12_{c}")
                t12p = T12[:].rearrange("p (b x) -> p b x", b=2)[:, :, 0:KV]
                nc.vector.tensor_tensor(t12p, scp, abp, mult)
                # L = T1 - T2 on Pool (plain TT; Pool is SBUF-only).
                # exp MUST run with scale=1.0: the ACT scale-multiplier
                # rounds at reduced precision, which at |L|~1e3 costs ~2e-2
                # absolute in the exponent (measured HW-vs-sim divergence).
                negL = work.tile([128, KV], FP32, tag="nL", name=f"nL{c}")
                nc.gpsimd.tensor_tensor(negL, T12[:, 304:304 + KV],
                                        T12[:, 0:KV], sub)
                negmx = small.tile([128, 1], FP32, tag="negmx")
                nc.vector.tensor_reduce(negmx, negL, mybir.AxisListType.X,
                                        mybir.AluOpType.max, negate=True)
                E = work.tile([128, KV], FP32, tag="E", name=f"E{c}")
                rs = small.tile([128, 1], FP32, tag="rs")
                nc.scalar.activation(E, negL, Exp, bias=negmx[:, 0:1],
                                     scale=1.0, accum_out=rs)
                rrs = small.tile([128, 1], FP32, tag="rrs", name=f"rrs{c}")
                nc.vector.reciprocal(rrs, rs)
                rrs_of[c] = rrs
                # G = E*C (DVE), H = E*S (Pool); softmax 1/rs is folded into
                # the node-major og copy (per-partition scale) later.
                nc.vector.tensor_tensor(t_G[c % GB][:, 0:KV], E, Cc, mult)
                nc.gpsimd.tensor_tensor(t_H[c % GB][:, 0:KV], E, Sc, mult)
                if cfg['debug'] and c == 0:
                    nc.sync.dma_start(dbg["d_SC0"][:], t_SC[0][:])
                    nc.sync.dma_start(dbg["d_KRT"][:], t_KRT[64:128, :])
                    nc.sync.dma_start(dbg["d_KIT"][:], t_KIT[64:128, :])
                    nc.sync.dma_start(dbg["d_T12"][:, 0:KV], T12[:, 0:KV])
                    nc.sync.dma_start(dbg["d_T12"][:, 304:304 + KV],
                                      T12[:, 304:304 + KV])
                    nc.sync.dma_start(dbg["d_nL"][:], negL[:])
                    nc.sync.dma_start(dbg["d_sml"][:, 0:1], negmx[:])
                    nc.sync.dma_start(dbg["d_sml"][:, 1:2], rs[:])
                    nc.sync.dma_start(dbg["d_sml"][:, 2:3], rrs[:])
                    nc.sync.dma_start(dbg["d_E"][:], E[:])
                    nc.sync.dma_start(dbg["d_G"][:], t_G[0][:])
                    nc.sync.dma_start(dbg["d_H"][:], t_H[0][:])

            def emit_tf(c):
                G = t_G[c % GB]
                H = t_H[c % GB]
                pst = ps_ght.tile([128, 768], FP16, tag="ght", name=f"pst{c}")
                for j in range(3):
                    jsl = slice(j * 128, (j + 1) * 128)
                    nc.tensor.transpose(pst[:, j * 128:(j + 1) * 128],
                                        G[:, jsl], t_ident)
                    nc.tensor.transpose(pst[:, 384 + j * 128:384 + (j + 1) * 128],
                                        H[:, jsl], t_ident)
                GHT = ghts.tile([128, 768], FP16, tag="ghts", name=f"GHT{c}")
                if DC > 0:
                    nc.vector.tensor_copy(GHT[:, 0:DC], pst[:, 0:DC])
                nc.scalar.activation(GHT[:, DC:768], pst[:, DC:768], Copy)
                # node-major finals: out[128 nodes, 64] = sum_j GHT_j^T @ VR_j
                og = psO[c // 8][:, (c % 8) * 64:(c % 8 + 1) * 64]
                for j in range(3):
                    w = 128 if j < 2 else 304 - 256
                    nc.tensor.matmul(og, GHT[0:w, j * 128:(j + 1) * 128],
                                     t_VRsb[0:w, j * 64:(j + 1) * 64],
                                     start=(j == 0), stop=False)
                    nc.tensor.matmul(og, GHT[0:w, 384 + j * 128:384 + (j + 1) * 128],
                                     t_VIsb[0:w, j * 64:(j + 1) * 64],
                                     start=False, stop=(j == 2))
                # per-chunk og copy applies the softmax normalization via a
                # per-partition (=node) scale; alternate DVE/ACT for balance
                g = c // 8
                if t_og_of[g] is None:
                    t_og_of[g] = work.tile([128, 512], FP32, tag="og",
                                           name=f"og{g}")
                t_og = t_og_of[g]
                osl = slice((c % 8) * 64, (c % 8 + 1) * 64)
                rrs = rrs_of[c]
                if c % 2 == 0:
                    nc.scalar.activation(t_og[:, osl], og, Copy,
                                         scale=rrs[:, 0:1])
                else:
                    nc.vector.tensor_scalar(t_og[:, osl], og, rrs[:, 0:1],
                                            None, mult)
                if cfg['debug'] and c == 0:
                    nc.sync.dma_start(dbg["d_GHT"][:], GHT[:])
                    nc.sync.dma_start(dbg["d_VRsb"][:], t_VRsb[:])
                    nc.sync.dma_start(dbg["d_VIsb"][:], t_VIsb[:])
                if cfg['debug'] and c == 7:
                    nc.sync.dma_start(dbg["d_og"][:], t_og[:])
                if c % 8 == 3:
                    nc.sync.dma_start(out_g[:, g * 512:g * 512 + 256],
                                      t_og[:, 0:256])
                if c % 8 == 7:
                    nc.sync.dma_start(out_g[:, g * 512 + 256:(g + 1) * 512],
                                      t_og[:, 256:512])

            vr_emitted = False

            def emit_vr_prep():
                # v_pot transposes: [K, d] layout in VRsb/VIsb. Emitted
                # after the first logits matmuls so the PE queue is not
                # blocked while the VRT copies drain.
                pvr = ps_ght.tile([128, 768], FP16, tag="ght", name="pvr")
                for j in range(3):
                    jsl = slice(j * 128, (j + 1) * 128)
                    nc.tensor.transpose(pvr[:, j * 64:(j + 1) * 64],
                                        t_VRT[:, jsl], t_ident[0:64, 0:64])
                    nc.tensor.transpose(pvr[:, 192 + j * 64:192 + (j + 1) * 64],
                                        t_VIn[:, jsl], t_ident[0:64, 0:64])
                nc.vector.tensor_copy(t_VRsb, pvr[:, 0:192])
                nc.vector.tensor_copy(t_VIsb, pvr[:, 192:384])

            ab = [None] * NCHUNK
            for c in range(NCHUNK):
                ab[c] = emit_ab(c)
                if not vr_emitted:
                    emit_vr_prep()
                    vr_emitted = True
                emit_softmax(c, ab[c])
                if c >= SH:
                    emit_tf(c - SH)
            for c in range(NCHUNK - SH, NCHUNK):
                emit_tf(c)

            ps_out.release()
            ps_ght.release()
            ps_ab.release()
    return nc


_NC_CACHE = {}


def _get_nc(cfg=None):
    key = tuple(sorted((cfg or {}).items()))
    if key not in _NC_CACHE:
        _install_bir_patch()
        _NC_CACHE[key] = _build_nc(cfg)
    return _NC_CACHE[key]


# ------------------------------------------------------------- host wrapper
def _kvecs(nk):
    kx = np.arange(0, nk[0] + 1)
    ky = np.arange(-nk[1], nk[1] + 1)
    kz = np.arange(-nk[2], nk[2] + 1)
    KX, KY, KZ = np.meshgrid(kx, ky, kz, indexing="ij")
    return np.stack([KX, KY, KZ], axis=-1).reshape(-1, 3).astype(np.float64)


def _prep_core_inputs(q, k, v, r, box):
    f = (r.astype(np.float64) / box[None, :]).astype(np.float32)  # [n,3]

    nk = [max(1, int(b)) for b in (box / DL).astype(np.int64)]
    kvs = _kvecs(nk)
    ksq = TWOPI ** 2 * ((kvs / box[None, :]) ** 2).sum(-1)
    valid = (ksq <= K_SQ_MAX) & (ksq > 0)
    kint = kvs[valid].astype(np.float32)  # [Kv,3] small integers
    Kv = kint.shape[0]
    assert Kv == KV, f"valid k-points {Kv} != {KV}"

    fh = f.astype(ml_dtypes.bfloat16).astype(np.float32)
    # residual scaled by 2^14 (exact) so fp16 operands avoid subnormals
    fl = ((f - fh) * 16384.0).astype(np.float16).astype(np.float32)

    kcolT = kint.T  # [3, KV]

    lhsb = np.zeros((48, N_NODE), np.float32)
    lhsb[0:3] = fh.T
    lhsb[3] = 1.0
    lhsb[4] = 1.0
    lhsb[5:8] = fh.T
    lhsb[32:35] = fh.T
    lhsb[35] = 1.0
    lhsb[36] = 1.0
    lhsb[37] = 1.0
    lhsb[38:41] = fh.T
    lhsb[41] = 1.0

    rhsb = np.zeros((48, KV), np.float32)
    rhsb[0:3] = kcolT
    rhsb[3] = MAGIC
    rhsb[4] = -MAGIC
    rhsb[5:8] = -kcolT
    rhsb[32:35] = kcolT
    rhsb[35] = 0.25
    rhsb[36] = MAGIC
    rhsb[37] = -MAGIC
    rhsb[38:41] = -kcolT
    rhsb[41] = -0.25

    lhsh = np.zeros((35, N_NODE), np.float32)
    lhsh[0:3] = fl.T
    lhsh[32:35] = fl.T
    rhsh = np.zeros((35, KV), np.float32)
    rhsh[0:3] = -kcolT / 16384.0
    rhsh[32:35] = -kcolT / 16384.0

    lhsh48 = np.zeros((48, N_NODE), np.float16)
    lhsh48[0:35] = lhsh.astype(np.float16)
    rhsh48 = np.zeros((48, KV), np.float16)
    rhsh48[0:35] = rhsh.astype(np.float16)
    lhs = np.concatenate([
        lhsb.astype(ml_dtypes.bfloat16).view(np.uint16),
        lhsh48.view(np.uint16)], axis=1)
    rhs = np.concatenate([
        rhsb.astype(ml_dtypes.bfloat16).view(np.uint16),
        rhsh48.view(np.uint16)], axis=1)

    kvc = np.concatenate([v, k], axis=1)  # [n,128]: v then k (see builder)
    kvcat = np.ascontiguousarray(
        kvc.reshape(NCHUNK, 128, 128).transpose(1, 0, 2).reshape(128, NCHUNK * 128)
    ).astype(np.float32)
    qt = np.ascontiguousarray(q.T).astype(np.float32)

    return {"lhs": lhs, "rhs": rhs, "kvcat": kvcat, "qt": qt}


def kernel(q_vector, k_vector, v_vector, positions, cell, batch):
    q_vector = np.asarray(q_vector)
    k_vector = np.asarray(k_vector)
    v_vector = np.asarray(v_vector)
    positions = np.asarray(positions)
    cell = np.asarray(cell)

    n = N_NODE
    boxes = np.diagonal(cell.reshape(-1, 3, 3), axis1=-2, axis2=-1)  # [B,3]

    in_maps = []
    for b in range(B):
        sl = slice(b * n, (b + 1) * n)
        in_maps.append(_prep_core_inputs(
            q_vector[sl], k_vector[sl], v_vector[sl], positions[sl],
            boxes[b].astype(np.float64)))

    nc = _get_nc()
    res = None
    last_err = None
    for _attempt in range(3):
        try:
            res = run_bass_kernel_spmd(nc, in_maps, list(range(B)))
            break
        except Exception as e:  # transient device states
            last_err = e
    if res is None:
        raise last_err

    out = np.empty((B * n, D), np.float32)
    for b in range(B):
        # out_g[p, g*512 + b8*64 + d] = out[g*1024 + b8*128 + p, d]
        og = res.results[b]["out_g"].reshape(128, 2, 8, D)  # [p, g, b8, d]
        out[b * n:(b + 1) * n] = og.transpose(1, 2, 0, 3).reshape(n, D)
    return out


if __name__ == "__main__":
    rng = np.random.default_rng(0)
    inputs = {
        "q_vector": rng.standard_normal((B * N_NODE, D), dtype=np.float32),
        "k_vector": rng.standard_normal((B * N_NODE, D), dtype=np.float32),
        "v_vector": rng.standard_normal((B * N_NODE, D), dtype=np.float32),
        "positions": rng.uniform(0, 20, (B * N_NODE, 3)).astype(np.float32),
        "cell": np.tile((np.eye(3, dtype=np.float32) * 20.0)[None], (B, 1, 1)),
        "batch": np.repeat(np.arange(B, dtype=np.int32), N_NODE),
    }
    o = kernel(**inputs)
    print("kernel ran, out", o.shape, o.dtype, float(np.abs(o).max()))



# revision 5
# speedup vs baseline: 1.1018x; 1.1018x over previous
"""Trainium2 Bass kernel for nn_EwaldPotential — v3.

Math per graph (all-real form of the complex reference):
  p[i,k] = (r[i,:]/box) @ kvecs[k,:]      (phase in cycles)
  C = cos(2*pi*p), S = sin(2*pi*p)
  KRT[d,k] = sum_i k[i,d] C[i,k]; KIT from S; VR/VI likewise from v
  A = q @ KRT, B = q @ KIT;  L = C*A - S*B
  P = softmax_k(L);  out = (P*C) @ VR - (P*S) @ VI

v3 design (all matmuls 1 cyc/row, exact-split precision):
 - phase: ONE fp16 11/13-row matmul per bank (f split fp16-hi/fp16-lo,
   magic rounding via the exact product 256*49152 = 1.5*2^23). Bank0
   accumulates frac(-p) so Sin gives -S; bank1 frac(p+1/4) gives C.
 - kpot: kv^T C split as kvH^T Ch + kvH^T Cl + kvl^T Ch (kvH/Ch are
   FREE bf16-truncation bitcast views of the fp32 tensors; kvl host
   fp16 residual; Cl/Sl one fused DVE TTR-sub per chunk, fp16 out,
   running in the otherwise-idle phase-1 DVE window).
 - logits: stationary q as bf16 hi/lo pair stacked on 128 partitions
   (exact to 2^-18) against fp16 KRTh duplicated on both halves; the
   fp16-residual KRTl correction runs as one fp8e4 DoubleRow matmul
   per output (0.5 cyc/row).
 - softmax: T12 = [SB | CA] one DVE pass ((-S)*(-B) = SB); fused DVE
   TTR: negL = SB-CA = -L with min-accum = -max(L); exp(scale=-1,
   bias=negmx) on ACT with rs accum; rrs folded into the Pool STTs
   G = (E*rrs)*C and -H = (E*rrs)*(-S), so the output needs no
   normalization copy: finals accumulate G@VR + (-H)@VI in PSUM and
   out DMAs straight from PSUM.
 - engine balance: ACT: Sin, exp, GHT tail; DVE: SCl sub (phase 1),
   T12, TTR, recip, GHT head; Pool: G/H STTs; PE: everything matmul.
"""
import json
import numpy as np
import ml_dtypes

import concourse.bass as bass
import concourse.tile as tile
from concourse import mybir
from concourse.bass_utils import run_bass_kernel_spmd

# ---------------------------------------------------------------- constants
B = 8
N_NODE = 2048
D = 64
DL = 4.0
TWOPI = 2.0 * np.pi
K_SQ_MAX = (TWOPI / DL) ** 2
KV = 297                    # valid k-point count
KVE = 298                   # even width for matmul outputs
NCHUNK = N_NODE // 128
MAGIC_A = 256.0             # MAGIC_A * MAGIC_B = 1.5 * 2^23 exactly
MAGIC_B = 49152.0

FP32 = mybir.dt.float32
FP32R = mybir.dt.float32r
BF16 = mybir.dt.bfloat16
FP16 = mybir.dt.float16
FP8E4 = mybir.dt.float8e4
U16 = mybir.dt.uint16
U8 = mybir.dt.uint8

# ------------------------------------------------- walrus wait-split patch
_ws_counter = [0]


def _split_waits_json(bir_bytes: bytes) -> bytes:
    d = json.loads(bir_bytes)
    changed = False
    for fn in d.get("functions", []):
        for blk in fn.get("blocks", []):
            out = []
            for ins in blk.get("instructions", []):
                si = ins.get("sync_info")
                ow = (si or {}).get("on_wait") or []
                if len(ow) > 1:
                    changed = True
                    for w in ow[:-1]:
                        _ws_counter[0] += 1
                        out.append({
                            "debug": ins.get("debug", 0),
                            "engine": ins.get("engine"),
                            "ins": [], "outs": [],
                            "name": f"I-wsplit{_ws_counter[0]}",
                            "opcode": "NoOp",
                            "sync_info": {"on_wait": [w], "on_update": []},
                        })
                    si["on_wait"] = [ow[-1]]
                out.append(ins)
            blk["instructions"] = out
    return json.dumps(d).encode() if changed else bir_bytes


def _install_bir_patch():
    import concourse.bass_utils as bu
    import concourse.bass2jax as b2j

    if getattr(b2j.compile_bir_kernel, "_wait_split", False):
        return
    orig = bu.compile_bir_kernel

    def wrapped(bir_json, tmpdir, neff_name="file.neff"):
        return orig(_split_waits_json(bir_json), tmpdir, neff_name)

    wrapped._wait_split = True
    b2j.compile_bir_kernel = wrapped
    bu.compile_bir_kernel = wrapped


# ------------------------------------------------------------ device kernel
def _build_nc(cfg=None):
    cfg = {**{'work_bufs': 4, 'ab_bufs': 2, 'ght_bufs': 2, 'gh_bufs': 4,
              'scl_bufs': 3, 'ph_bufs': 3, 'shift': 2, 'ght_dve_cols': 0,
              'logits_corr': 'dr', 'og_eng': 'act', 'og_dma': 4,
              'out_bufs': 2, 'kpot_m3': True, 'debug': False}, **(cfg or {})}
    nc = bass.Bass("TRN2")
    lhs = nc.dram_tensor("lhs", [48, N_NODE], U16, kind="ExternalInput")
    rhs = nc.dram_tensor("rhs", [48, KV], U16, kind="ExternalInput")
    # kvcat rows 0:64 = v dims, 64:128 = k dims; chunk-major free axis
    kvcat = nc.dram_tensor("kvcat", [128, N_NODE], FP32, kind="ExternalInput")
    kvl = nc.dram_tensor("kvl", [128, N_NODE], U16, kind="ExternalInput")
    qs = nc.dram_tensor("qs", [128, N_NODE], U16, kind="ExternalInput")
    q8z = nc.dram_tensor("q8z", [64, NCHUNK * 256], U8, kind="ExternalInput")
    # out_g[p, g*512 + (c%8)*64 + d] = out[c*128 + p, d],  g = c//8
    out_g = nc.dram_tensor("out_g", [128, 1024], FP32, kind="ExternalOutput")
    rs_out = nc.dram_tensor("rs_out", [128, NCHUNK], FP32, kind="ExternalOutput")
    dbg = {}
    if cfg['debug']:
        for nm, shape, dt in (("d_SC0", [128, 608], FP32),
                              ("d_SCl0", [128, 608], FP16),
                              ("d_KC", [128, KVE], FP32),
                              ("d_KS", [128, KVE], FP32),
                              ("d_KRTh", [128, KVE], FP16),
                              ("d_KITh", [128, KVE], FP16),
                              ("d_KRTl8", [64, 2 * KVE], U8),
                              ("d_AB", [128, 1024], FP32),
                              ("d_nL", [128, KV], FP32),
                              ("d_sml", [128, 3], FP32),
                              ("d_E", [128, KV], FP32),
                              ("d_G", [128, 384], FP16),
                              ("d_nH", [128, 384], FP16),
                              ("d_VRsb", [128, 192], FP16),
                              ("d_VIsb", [128, 192], FP16)):
            dbg[nm] = nc.dram_tensor(nm, shape, dt, kind="ExternalOutput")

    Sin = mybir.ActivationFunctionType.Sin
    Exp = mybir.ActivationFunctionType.Exp
    Copy = mybir.ActivationFunctionType.Copy
    mult = mybir.AluOpType.mult
    sub = mybir.AluOpType.subtract
    amin = mybir.AluOpType.min
    amax = mybir.AluOpType.max
    DR = mybir.MatmulPerfMode.DoubleRow

    GB = cfg['gh_bufs']
    NH = N_NODE // 2
    DC = cfg['ght_dve_cols']

    with tile.TileContext(nc) as tc:
        with tc.tile_pool(name="consts", bufs=1) as consts, \
             tc.tile_pool(name="work", bufs=cfg['work_bufs']) as work, \
             tc.tile_pool(name="scl", bufs=cfg['scl_bufs']) as sclp, \
             tc.tile_pool(name="ghts", bufs=cfg['ght_bufs']) as ghts, \
             tc.tile_pool(name="small", bufs=4) as small:

            t_lhs = consts.tile([48, N_NODE], U16, tag="lhs")
            t_rhs = consts.tile([48, KV], U16, tag="rhs")
            t_kv = consts.tile([128, N_NODE], FP32, tag="kv")
            t_kvl = consts.tile([128, N_NODE], U16, tag="kvl")
            t_qs = consts.tile([128, N_NODE], U16, tag="qs")
            t_q8z = consts.tile([64, NCHUNK * 256], U8, tag="q8z")
            # phase weights first so phase matmuls start ASAP
            nc.sync.dma_start(t_lhs[0:16, :], lhs[0:16, :])
            nc.sync.dma_start(t_rhs, rhs[:])
            nc.sync.dma_start(t_lhs[16:48, :], lhs[16:48, :])
            NQ = N_NODE // 4
            for i in range(4):
                s = slice(i * NQ, (i + 1) * NQ)
                nc.sync.dma_start(t_kv[:, s], kvcat[:, s])
                nc.sync.dma_start(t_kvl[:, s], kvl[:, s])
            nc.sync.dma_start(t_qs, qs[:])
            nc.sync.dma_start(t_q8z, q8z[:])

            lb = t_lhs[0:48, :].bitcast(FP16)
            rb = t_rhs[0:48, :].bitcast(FP16)

            # kv bf16-hi view: fp32 high half-words (odd u16 index)
            kv_bf = t_kv[:].bitcast(BF16).rearrange(
                "p (c d two) -> p c d two", c=NCHUNK, two=2)
            kvl_r = t_kvl[:].bitcast(FP16).rearrange(
                "p (c d) -> p c d", c=NCHUNK)
            qs_r = t_qs[:].bitcast(BF16).rearrange(
                "p (c n) -> p c n", c=NCHUNK)
            q8_r = t_q8z[:].bitcast(FP8E4).rearrange(
                "p (c two n) -> p c two n", c=NCHUNK, two=2)

            from concourse.masks import make_identity
            t_ident = consts.tile([128, 128], FP16, tag="ident")
            make_identity(nc, t_ident)
            # PE warmup: burn the p-state ramp during the input DMAs
            with tc.tile_pool(name="ps_warm", bufs=1, space="PSUM") as ps_w:
                pw = ps_w.tile([128, 128], FP16, tag="warm")
                for _ in range(14):
                    nc.tensor.transpose(pw, t_ident, t_ident)

            # S|C fp32 slabs: [-S at 0:304 | C at 304:608]
            t_SC = [consts.tile([128, 608], FP32, tag=f"SC{c}",
                                name=f"SCt{c}") for c in range(NCHUNK)]
            for c in range(NCHUNK):
                pad = t_SC[c][:].rearrange(
                    "p (b x) -> p b x", b=2)[:, :, KV:304]
                nc.vector.memset(pad, 0.0)
            # fp16 residual slabs [Sl | Cl] (ring)
            t_SCl = [sclp.tile([128, 608], FP16, tag="SCl",
                               name=f"SCl{i}") for i in range(cfg['scl_bufs'])]
            for i in range(cfg['scl_bufs']):
                pad = t_SCl[i][:].rearrange(
                    "p (b x) -> p b x", b=2)[:, :, KV:304]
                nc.vector.memset(pad, 0.0)

            # G / -H rings [128, 384] fp16, pads zeroed once
            t_G = [consts.tile([128, 384], FP16, tag=f"G{i}", name=f"Gt{i}")
                   for i in range(GB)]
            t_nH = [consts.tile([128, 384], FP16, tag=f"H{i}", name=f"Ht{i}")
                    for i in range(GB)]
            for i in range(GB):
                nc.vector.memset(t_G[i][:, KV:384], 0.0)
                nc.vector.memset(t_nH[i][:, KV:384], 0.0)

            t_dummy = small.tile([128, 1], FP32, tag="dummy")
            t_rs = consts.tile([128, NCHUNK], FP32, tag="rs_all")

            # ---------------- phase 1: trig + k_pot/v_pot accumulation
            # per chunk: 2 phase mms -> Sin -> TTR residual split (DVE) and
            # 6 kpot mms; kpot(c-1) emitted after phase(c) so the PE never
            # head-of-line blocks on the Sin chain.
            def sc_views(c):
                sc = t_SC[c]
                ch = sc[:].bitcast(BF16).rearrange(
                    "p (b x two) -> p b x two", b=2, two=2)[:, 1, 0:KVE, 1]
                sh = sc[:].bitcast(BF16).rearrange(
                    "p (b x two) -> p b x two", b=2, two=2)[:, 0, 0:KVE, 1]
                return sh, ch

            def emit_kpot_main(c):
                shv, chv = sc_views(c)
                kvh = kv_bf[:, c, :, 1]
                # main: kvH^T Ch / kvH^T (-S)h
                nc.tensor.matmul(psKC, kvh, chv,
                                 start=(c == 0), stop=False)
                nc.tensor.matmul(psKS, kvh, shv,
                                 start=(c == 0), stop=False)
                if cfg['kpot_m3']:
                    # m3: kvl^T Ch / kvl^T Sh
                    nc.tensor.matmul(psKC, kvl_r[:, c, :], chv,
                                     start=False, stop=False)
                    nc.tensor.matmul(psKS, kvl_r[:, c, :], shv,
                                     start=False, stop=False)

            def emit_kpot_m2(c):
                # m2: kvH^T Cl / kvH^T Sl (shifted: SCl comes off the
                # Sin->TTR chain, so give it 2 chunks of slack)
                sl_scl = t_SCl[c % cfg['scl_bufs']]
                kvh = kv_bf[:, c, :, 1]
                nc.tensor.matmul(psKC, kvh, sl_scl[:, 304:304 + KVE],
                                 start=False, stop=(c == NCHUNK - 1))
                nc.tensor.matmul(psKS, kvh, sl_scl[:, 0:KVE],
                                 start=False, stop=(c == NCHUNK - 1))

            with tc.tile_pool(name="ps_ph", bufs=cfg['ph_bufs'],
                              space="PSUM") as ps_ph, \
                 tc.tile_pool(name="ps_kc", bufs=1, space="PSUM") as ps_kc:
                psKC = ps_kc.tile([128, KVE], FP32, tag="KC")
                psKS = ps_kc.tile([128, KVE], FP32, tag="KS")
                for c in range(NCHUNK):
                    sl = slice(c * 128, (c + 1) * 128)
                    pSC = ps_ph.tile([128, 1024], FP32, tag="pSC")
                    # bank1 (C): rows 0:13; bank0 (-S): rows 32:43
                    nc.tensor.matmul(pSC[:, 512:512 + KV], lb[0:13, sl],
                                     rb[0:13, :], start=True, stop=True,
                                     tile_position=(0, 0))
                    nc.tensor.matmul(pSC[:, 0:KV], lb[32:43, sl],
                                     rb[32:43, :], start=True, stop=True,
                                     tile_position=(32, 0))
                    pin = pSC[:].rearrange("p (b x) -> p b x", b=2)[:, :, 0:KV]
                    sout = t_SC[c][:].rearrange(
                        "p (b x) -> p b x", b=2)[:, :, 0:KV]
                    nc.scalar.activation(sout, pin, Sin, scale=-TWOPI)
                    # residual split: [Sl|Cl] = SC - bf16trunc(SC), fp16 out
                    scl = t_SCl[c % cfg['scl_bufs']]
                    sc_b = t_SC[c][:].bitcast(BF16).rearrange(
                        "p (b x two) -> p b x two", b=2, two=2)[:, :, 0:KV, 1]
                    nc.vector.tensor_tensor(
                        scl[:].rearrange("p (b x) -> p b x", b=2)[:, :, 0:KV],
                        sout, sc_b, sub)
                    if c >= 1:
                        emit_kpot_main(c - 1)
                    if c >= 2:
                        emit_kpot_m2(c - 2)
                emit_kpot_main(NCHUNK - 1)
                emit_kpot_m2(NCHUNK - 2)
                emit_kpot_m2(NCHUNK - 1)

                # ---- k_pot / v_pot staging (one-time) --------------------
                # KRTh/KITh' fp16 duplicated on both 64-partition halves
                t_KRTh = consts.tile([128, KVE], FP16, tag="KRTh")
                t_KITh = consts.tile([128, KVE], FP16, tag="KITh")
                nc.scalar.activation(t_KITh[0:64, :], psKS[64:128, :], Copy)
                nc.scalar.activation(t_KITh[64:128, :], psKS[64:128, :], Copy)
                nc.vector.tensor_copy(t_KRTh[0:64, :], psKC[64:128, :])
                nc.vector.tensor_copy(t_KRTh[64:128, :], psKC[64:128, :])
                # fp8 residuals (x16) for the DoubleRow corrections
                t_KRTl8 = consts.tile([64, 2 * KVE], FP8E4, tag="KRTl8")
                t_KITl8 = consts.tile([64, 2 * KVE], FP8E4, tag="KITl8")
                nc.vector.memset(t_KRTl8[:, KVE:], 0.0)
                nc.vector.memset(t_KITl8[:, KVE:], 0.0)
                t_Ltmp = consts.tile([64, 2 * KVE], FP32, tag="Ltmp")
                nc.vector.tensor_tensor(t_Ltmp[:, 0:KVE], psKC[64:128, :],
                                        t_KRTh[0:64, :], sub)
                nc.vector.tensor_tensor(t_Ltmp[:, KVE:], psKS[64:128, :],
                                        t_KITh[0:64, :], sub)
                nc.vector.tensor_scalar(t_KRTl8[:, 0:KVE], t_Ltmp[:, 0:KVE],
                                        16.0, None, mult)
                nc.vector.tensor_scalar(t_KITl8[:, 0:KVE], t_Ltmp[:, KVE:],
                                        16.0, None, mult)
                # v_pot: VR = psKC[0:64], VI = -psKS[0:64]
                t_VRT = consts.tile([64, 384], FP16, tag="VRT")
                t_VIT = consts.tile([64, 384], FP16, tag="VIT")
                nc.vector.memset(t_VRT[:, KV:384], 0.0)
                nc.vector.memset(t_VIT[:, KV:384], 0.0)
                nc.scalar.activation(t_VRT[:, 0:KV], psKC[0:64, 0:KV], Copy)
                nc.scalar.activation(t_VIT[:, 0:KV], psKS[0:64, 0:KV], Copy,
                                     scale=-1.0)
                if cfg['debug']:
                    t_dKC = consts.tile([128, KVE], FP32, tag="dKC")
                    t_dKS = consts.tile([128, KVE], FP32, tag="dKS")
                    nc.vector.tensor_copy(t_dKC, psKC[:])
                    nc.vector.tensor_copy(t_dKS, psKS[:])
                    nc.sync.dma_start(dbg["d_KC"][:], t_dKC[:])
                    nc.sync.dma_start(dbg["d_KS"][:], t_dKS[:])

            t_VRsb = consts.tile([128, 192], FP16, tag="VRsb")
            t_VIsb = consts.tile([128, 192], FP16, tag="VIsb")

            ps_ab = tc.alloc_tile_pool(name="ps_ab", bufs=cfg['ab_bufs'],
                                       space="PSUM")
            ps_ght = tc.alloc_tile_pool(name="ps_ght", bufs=cfg['ght_bufs'],
                                        space="PSUM")
            ps_out = tc.alloc_tile_pool(name="ps_out", bufs=cfg['out_bufs'],
                                        space="PSUM")

            # ---------------- phase 2: logits, softmax, inverse transform
            psO = [None] * 2
            t_og_of = [None] * 2
            E_of = [None] * NCHUNK

            def emit_ab_main(c):
                if c % 8 == 0:
                    psO[c // 8] = ps_out.tile([128, 512], FP32, tag="O",
                                              name=f"Ot{c//8}")
                psAB = ps_ab.tile([128, 1024], FP32, tag="AB",
                                  name=f"psAB{c}")
                # bank0 = q^T KIT' (= -B), bank1 = q^T KRT (= A)
                nc.tensor.matmul(psAB[:, 0:KVE], qs_r[:, c, :], t_KITh,
                                 start=True, stop=(cfg['logits_corr'] != 'dr'))
                nc.tensor.matmul(psAB[:, 512:512 + KVE], qs_r[:, c, :],
                                 t_KRTh,
                                 start=True, stop=(cfg['logits_corr'] != 'dr'))
                return psAB

            def emit_ab_corr(c, psAB):
                if cfg['logits_corr'] == 'dr':
                    q8c = q8_r[:, c, :, :]
                    nc.tensor.matmul(
                        psAB[:, 0:KVE], q8c,
                        t_KITl8[:].rearrange("p (two x) -> p two x", two=2),
                        start=False, stop=True, perf_mode=DR,
                        skip_group_check=True)
                    nc.tensor.matmul(
                        psAB[:, 512:512 + KVE], q8c,
                        t_KRTl8[:].rearrange("p (two x) -> p two x", two=2),
                        start=False, stop=True, perf_mode=DR,
                        skip_group_check=True)

            def emit_softmax(c, psAB):
                Cc = t_SC[c][:, 304:304 + KV]
                nSc = t_SC[c][:, 0:KV]
                scp = t_SC[c][:].rearrange("p (b x) -> p b x", b=2)[:, :, 0:KV]
                abp = psAB[:].rearrange("p (b x) -> p b x", b=2)[:, :, 0:KV]
                # T12 = [(-S)(-B) | C*A] = [SB | CA] in one DVE pass
                T12 = work.tile([128, 608], FP32, tag="T12", name=f"T12_{c}")
                t12p = T12[:].rearrange("p (b x) -> p b x", b=2)[:, :, 0:KV]
                nc.vector.tensor_tensor(t12p, scp, abp, mult)
                # L = CA - SB (DVE; keeps the T12->L->max chain one-queue)
                Lt = work.tile([128, KV], FP32, tag="nL", name=f"nL{c}")
                nc.vector.tensor_tensor(Lt, T12[:, 304:304 + KV],
                                        T12[:, 0:KV], sub)
                negmx = small.tile([128, 1], FP32, tag="negmx")
                nc.vector.tensor_reduce(negmx, Lt, mybir.AxisListType.X,
                                        amax, negate=True)
                # E = exp(L - max L); rs lands in rs_all - the softmax
                # normalization (divide by rs) happens on the host
                E = work.tile([128, KV], FP32, tag="E", name=f"E{c}")
                nc.scalar.activation(E, Lt, Exp, bias=negmx[:, 0:1],
                                     scale=1.0, accum_out=t_rs[:, c:c + 1])
                E_of[c] = E
                if cfg['debug'] and c == 0:
                    nc.sync.dma_start(dbg["d_SC0"][:], t_SC[0][:])
                    nc.sync.dma_start(dbg["d_SCl0"][:], t_SCl[0][:])
                    nc.sync.dma_start(dbg["d_KRTh"][:], t_KRTh[:])
                    nc.sync.dma_start(dbg["d_KITh"][:], t_KITh[:])
                    nc.sync.dma_start(dbg["d_KRTl8"][:],
                                      t_KRTl8[:].bitcast(U8))
                    t_dAB = consts.tile([128, 1024], FP32, tag="dAB")
                    nc.vector.tensor_copy(t_dAB, psAB[:])
                    nc.sync.dma_start(dbg["d_AB"][:], t_dAB[:])
                    nc.sync.dma_start(dbg["d_nL"][:], Lt[:])
                    nc.sync.dma_start(dbg["d_sml"][:, 0:1], negmx[:])
                    nc.sync.dma_start(dbg["d_sml"][:, 1:2], rs[:])
                    nc.sync.dma_start(dbg["d_sml"][:, 2:3], rrs[:])
                    nc.sync.dma_start(dbg["d_E"][:], E[:])


            def emit_gh(c):
                # G = E*C (DVE), -H = E*(-S) (Pool); 1/rs folds into og.
                # Emitted one chunk late so the DVE/Pool queues never wait
                # on the exp of the chunk they just fed.
                E = E_of[c]
                Cc = t_SC[c][:, 304:304 + KV]
                nSc = t_SC[c][:, 0:KV]
                nc.gpsimd.tensor_tensor(t_G[c % GB][:, 0:KV], E, Cc, mult)
                nc.gpsimd.tensor_tensor(t_nH[c % GB][:, 0:KV], E, nSc, mult)

            def emit_tf(c):
                G = t_G[c % GB]
                nH = t_nH[c % GB]
                pst = ps_ght.tile([128, 768], FP16, tag="ght", name=f"pst{c}")
                for j in range(3):
                    jsl = slice(j * 128, (j + 1) * 128)
                    nc.tensor.transpose(pst[:, j * 128:(j + 1) * 128],
                                        G[:, jsl], t_ident)
                    nc.tensor.transpose(
                        pst[:, 384 + j * 128:384 + (j + 1) * 128],
                        nH[:, jsl], t_ident)
                GHT = ghts.tile([128, 768], FP16, tag="ghts", name=f"GHT{c}")
                if DC > 0:
                    nc.vector.tensor_copy(GHT[:, 0:DC], pst[:, 0:DC])
                if DC < 768:
                    nc.scalar.activation(GHT[:, DC:768], pst[:, DC:768], Copy)
                og = psO[c // 8][:, (c % 8) * 64:(c % 8 + 1) * 64]
                for j in range(3):
                    w = 128 if j < 2 else KV - 256
                    nc.tensor.matmul(og, GHT[0:w, j * 128:(j + 1) * 128],
                                     t_VRsb[0:w, j * 64:(j + 1) * 64],
                                     start=(j == 0), stop=False)
                    nc.tensor.matmul(
                        og, GHT[0:w, 384 + j * 128:384 + (j + 1) * 128],
                        t_VIsb[0:w, j * 64:(j + 1) * 64],
                        start=False, stop=(j == 2))
                # PSUM cannot source a DMA: stage og in SBUF. Plain copy
                # (host divides by rs), batched per 2 chunks at odd c.
                g = c // 8
                if t_og_of[g] is None:
                    t_og_of[g] = work.tile([128, 512], FP32, tag="og",
                                           name=f"og{g}")
                t_og = t_og_of[g]
                if c % 2 == 1:
                    osl = slice((c % 8 - 1) * 64, (c % 8 + 1) * 64)
                    ogs = psO[g][:, osl]
                    if cfg['og_eng'] == 'act':
                        nc.scalar.activation(t_og[:, osl], ogs, Copy)
                    else:
                        nc.vector.tensor_copy(t_og[:, osl], ogs)
                if cfg['debug'] and c == 0:
                    nc.sync.dma_start(dbg["d_VRsb"][:], t_VRsb[:])
                    nc.sync.dma_start(dbg["d_VIsb"][:], t_VIsb[:])
                q = cfg['og_dma']
                step = 8 // q
                if (c % 8) % step == step - 1:
                    i0 = ((c % 8) // step) * 64 * step
                    i1 = i0 + 64 * step
                    nc.sync.dma_start(out_g[:, g * 512 + i0:g * 512 + i1],
                                      t_og[:, i0:i1])

            vr_emitted = False

            def emit_vr_prep():
                pvr = ps_ght.tile([128, 768], FP16, tag="ght", name="pvr")
                for j in range(3):
                    jsl = slice(j * 128, (j + 1) * 128)
                    nc.tensor.transpose(pvr[:, j * 64:(j + 1) * 64],
                                        t_VRT[:, jsl], t_ident[0:64, 0:64])
                    nc.tensor.transpose(
                        pvr[:, 192 + j * 64:192 + (j + 1) * 64],
                        t_VIT[:, jsl], t_ident[0:64, 0:64])
                nc.vector.tensor_copy(t_VRsb, pvr[:, 0:192])
                nc.vector.tensor_copy(t_VIsb, pvr[:, 192:384])

            SH = cfg['shift']
            psAB_of = [None] * NCHUNK
            for c in range(NCHUNK):
                psAB_of[c] = emit_ab_main(c)
                if c >= 1:
                    emit_ab_corr(c - 1, psAB_of[c - 1])
                if not vr_emitted:
                    emit_vr_prep()
                    vr_emitted = True
                if c >= 1:
                    emit_softmax(c - 1, psAB_of[c - 1])
                if c >= 2:
                    emit_gh(c - 2)
                if c >= 2 + SH:
                    emit_tf(c - 2 - SH)
            emit_ab_corr(NCHUNK - 1, psAB_of[NCHUNK - 1])
            emit_softmax(NCHUNK - 1, psAB_of[NCHUNK - 1])
            emit_gh(NCHUNK - 2)
            emit_gh(NCHUNK - 1)
            for c in range(NCHUNK - 2 - SH, NCHUNK):
                emit_tf(c)
            nc.sync.dma_start(rs_out[:], t_rs[:])

            ps_out.release()
            ps_ght.release()
            ps_ab.release()
    return nc


_NC_CACHE = {}


def _get_nc(cfg=None):
    key = tuple(sorted((cfg or {}).items()))
    if key not in _NC_CACHE:
        _install_bir_patch()
        _NC_CACHE[key] = _build_nc(cfg)
    return _NC_CACHE[key]


# ------------------------------------------------------------- host wrapper
def _kvecs(nk):
    kx = np.arange(0, nk[0] + 1)
    ky = np.arange(-nk[1], nk[1] + 1)
    kz = np.arange(-nk[2], nk[2] + 1)
    KX, KY, KZ = np.meshgrid(kx, ky, kz, indexing="ij")
    return np.stack([KX, KY, KZ], axis=-1).reshape(-1, 3).astype(np.float64)


def _bf16_trunc(x):
    """bf16 truncation (high 16 bits) of fp32, as fp32."""
    xv = np.ascontiguousarray(x, dtype=np.float32).view(np.uint32)
    return (xv & 0xFFFF0000).view(np.float32)


def _prep_core_inputs(q, k, v, r, box):
    f = (r.astype(np.float64) / box[None, :])  # [n,3] in [0,1)

    nk = [max(1, int(b)) for b in (box / DL).astype(np.int64)]
    kvs = _kvecs(nk)
    ksq = TWOPI ** 2 * ((kvs / box[None, :]) ** 2).sum(-1)
    valid = (ksq <= K_SQ_MAX) & (ksq > 0)
    kint = kvs[valid]  # [KV,3] small integers
    assert kint.shape[0] == KV, f"valid k-points {kint.shape[0]} != {KV}"
    kcolT = kint.T  # [3, KV]

    fh = f.astype(np.float16).astype(np.float64)
    fl = (f - fh).astype(np.float16).astype(np.float64)

    lhsb = np.zeros((48, N_NODE), np.float16)
    rhsb = np.zeros((48, KV), np.float16)
    # bank1 (C): accumulate p_h + 1/4 -> round -> -p_h - p_l - 1/4
    lhsb[0:3] = fh.T
    lhsb[3] = 1.0
    lhsb[4] = MAGIC_A
    lhsb[5] = MAGIC_A
    lhsb[6:9] = fh.T
    lhsb[9:12] = fl.T
    lhsb[12] = 1.0
    rhsb[0:3] = kcolT
    rhsb[3] = 0.25
    rhsb[4] = MAGIC_B
    rhsb[5] = -MAGIC_B
    rhsb[6:9] = -kcolT
    rhsb[9:12] = -kcolT
    rhsb[12] = -0.25
    # bank0 (-S): accumulate -p_h -> round -> +p_h + p_l
    lhsb[32:35] = fh.T
    lhsb[35] = MAGIC_A
    lhsb[36] = MAGIC_A
    lhsb[37:40] = fh.T
    lhsb[40:43] = fl.T
    rhsb[32:35] = -kcolT
    rhsb[35] = MAGIC_B
    rhsb[36] = -MAGIC_B
    rhsb[37:40] = kcolT
    rhsb[40:43] = kcolT

    # kv concat: rows 0:64 v, 64:128 k; chunk-major free axis
    kvc = np.concatenate([v, k], axis=1)  # [n,128]
    kvcat = np.ascontiguousarray(
        kvc.reshape(NCHUNK, 128, 128).transpose(1, 0, 2).reshape(128, N_NODE)
    ).astype(np.float32)
    kvl16 = (kvcat - _bf16_trunc(kvcat)).astype(np.float16)

    # q bf16 hi/lo pair, stacked on partitions: rows 0:64 hi, 64:128 lo
    qT = np.ascontiguousarray(q.T).astype(np.float32)  # [64, n]
    qh = qT.astype(ml_dtypes.bfloat16)
    ql = (qT - qh.astype(np.float32)).astype(ml_dtypes.bfloat16)
    qs16 = np.concatenate([qh.view(np.uint16), ql.view(np.uint16)], axis=0)

    # fp8 q/16 with zero slot1 per chunk: [64, c, 2, 128]
    q8 = (qT / 16.0).astype(ml_dtypes.float8_e4m3)
    q8z = np.zeros((64, NCHUNK, 2, 128), ml_dtypes.float8_e4m3)
    q8z[:, :, 0, :] = q8.reshape(64, NCHUNK, 128)

    return {"lhs": lhsb.view(np.uint16), "rhs": rhsb.view(np.uint16),
            "kvcat": kvcat, "kvl": kvl16.view(np.uint16),
            "qs": qs16, "q8z": q8z.reshape(64, -1).view(np.uint8)}


def kernel(q_vector, k_vector, v_vector, positions, cell, batch):
    q_vector = np.asarray(q_vector)
    k_vector = np.asarray(k_vector)
    v_vector = np.asarray(v_vector)
    positions = np.asarray(positions)
    cell = np.asarray(cell)

    n = N_NODE
    boxes = np.diagonal(cell.reshape(-1, 3, 3), axis1=-2, axis2=-1)  # [B,3]

    in_maps = []
    for b in range(B):
        sl = slice(b * n, (b + 1) * n)
        in_maps.append(_prep_core_inputs(
            q_vector[sl], k_vector[sl], v_vector[sl], positions[sl],
            boxes[b].astype(np.float64)))

    nc = _get_nc()
    res = None
    last_err = None
    for _attempt in range(3):
        try:
            res = run_bass_kernel_spmd(nc, in_maps, list(range(B)))
            break
        except Exception as e:  # transient device states
            last_err = e
    if res is None:
        raise last_err

    out = np.empty((B * n, D), np.float32)
    for b in range(B):
        og = res.results[b]["out_g"].reshape(128, 2, 8, D)  # [p, g, c8, d]
        o = og.transpose(1, 2, 0, 3).reshape(n, D)
        rs = res.results[b]["rs_out"].T.reshape(n, 1)  # [c*128+p]
        out[b * n:(b + 1) * n] = o / rs
    return out


if __name__ == "__main__":
    rng = np.random.default_rng(0)
    inputs = {
        "q_vector": rng.standard_normal((B * N_NODE, D), dtype=np.float32),
        "k_vector": rng.standard_normal((B * N_NODE, D), dtype=np.float32),
        "v_vector": rng.standard_normal((B * N_NODE, D), dtype=np.float32),
        "positions": rng.uniform(0, 20, (B * N_NODE, 3)).astype(np.float32),
        "cell": np.tile((np.eye(3, dtype=np.float32) * 20.0)[None], (B, 1, 1)),
        "batch": np.repeat(np.arange(B, dtype=np.int32), N_NODE),
    }
    o = kernel(**inputs)
    print("kernel ran, out", o.shape, o.dtype, float(np.abs(o).max()))


# revision 6
# speedup vs baseline: 1.1229x; 1.0191x over previous
"""Trainium2 Bass kernel for nn_EwaldPotential — v3.

Math per graph (all-real form of the complex reference):
  p[i,k] = (r[i,:]/box) @ kvecs[k,:]      (phase in cycles)
  C = cos(2*pi*p), S = sin(2*pi*p)
  KRT[d,k] = sum_i k[i,d] C[i,k]; KIT from S; VR/VI likewise from v
  A = q @ KRT, B = q @ KIT;  L = C*A - S*B
  P = softmax_k(L);  out = (P*C) @ VR - (P*S) @ VI

v3 design (all matmuls 1 cyc/row, exact-split precision):
 - phase: ONE fp16 11/13-row matmul per bank (f split fp16-hi/fp16-lo,
   magic rounding via the exact product 256*49152 = 1.5*2^23). Bank0
   accumulates frac(-p) so Sin gives -S; bank1 frac(p+1/4) gives C.
 - kpot: kv^T C split as kvH^T Ch + kvH^T Cl + kvl^T Ch (kvH/Ch are
   FREE bf16-truncation bitcast views of the fp32 tensors; kvl host
   fp16 residual; Cl/Sl one fused DVE TTR-sub per chunk, fp16 out,
   running in the otherwise-idle phase-1 DVE window).
 - logits: stationary q as bf16 hi/lo pair stacked on 128 partitions
   (exact to 2^-18) against fp16 KRTh duplicated on both halves; the
   fp16-residual KRTl correction runs as one fp8e4 DoubleRow matmul
   per output (0.5 cyc/row).
 - softmax: T12 = [SB | CA] one DVE pass ((-S)*(-B) = SB); fused DVE
   TTR: negL = SB-CA = -L with min-accum = -max(L); exp(scale=-1,
   bias=negmx) on ACT with rs accum; rrs folded into the Pool STTs
   G = (E*rrs)*C and -H = (E*rrs)*(-S), so the output needs no
   normalization copy: finals accumulate G@VR + (-H)@VI in PSUM and
   out DMAs straight from PSUM.
 - engine balance: ACT: Sin, exp, GHT tail; DVE: SCl sub (phase 1),
   T12, TTR, recip, GHT head; Pool: G/H STTs; PE: everything matmul.
"""
import json
import numpy as np
import ml_dtypes

import concourse.bass as bass
import concourse.tile as tile
from concourse import mybir
from concourse.bass_utils import run_bass_kernel_spmd

# ---------------------------------------------------------------- constants
B = 8
N_NODE = 2048
D = 64
DL = 4.0
TWOPI = 2.0 * np.pi
K_SQ_MAX = (TWOPI / DL) ** 2
KV = 297                    # valid k-point count
KVE = 298                   # even width for matmul outputs
NCHUNK = N_NODE // 128
MAGIC_A = 256.0             # MAGIC_A * MAGIC_B = 1.5 * 2^23 exactly
MAGIC_B = 49152.0

FP32 = mybir.dt.float32
FP32R = mybir.dt.float32r
BF16 = mybir.dt.bfloat16
FP16 = mybir.dt.float16
FP8E4 = mybir.dt.float8e4
U16 = mybir.dt.uint16
U8 = mybir.dt.uint8

# ------------------------------------------------- walrus wait-split patch
_ws_counter = [0]


def _split_waits_json(bir_bytes: bytes) -> bytes:
    d = json.loads(bir_bytes)
    changed = False
    for fn in d.get("functions", []):
        for blk in fn.get("blocks", []):
            out = []
            for ins in blk.get("instructions", []):
                si = ins.get("sync_info")
                ow = (si or {}).get("on_wait") or []
                if len(ow) > 1:
                    changed = True
                    for w in ow[:-1]:
                        _ws_counter[0] += 1
                        out.append({
                            "debug": ins.get("debug", 0),
                            "engine": ins.get("engine"),
                            "ins": [], "outs": [],
                            "name": f"I-wsplit{_ws_counter[0]}",
                            "opcode": "NoOp",
                            "sync_info": {"on_wait": [w], "on_update": []},
                        })
                    si["on_wait"] = [ow[-1]]
                out.append(ins)
            blk["instructions"] = out
    return json.dumps(d).encode() if changed else bir_bytes


def _install_bir_patch():
    import concourse.bass_utils as bu
    import concourse.bass2jax as b2j

    if getattr(b2j.compile_bir_kernel, "_wait_split", False):
        return
    orig = bu.compile_bir_kernel

    def wrapped(bir_json, tmpdir, neff_name="file.neff"):
        return orig(_split_waits_json(bir_json), tmpdir, neff_name)

    wrapped._wait_split = True
    b2j.compile_bir_kernel = wrapped
    bu.compile_bir_kernel = wrapped


# ------------------------------------------------------------ device kernel
def _build_nc(cfg=None):
    cfg = {**{'work_bufs': 4, 'ab_bufs': 2, 'ght_bufs': 2, 'gh_bufs': 4,
              'scl_bufs': 3, 'ph_bufs': 3, 'shift': 2, 'ght_dve_cols': 16,
              'logits_corr': 'dr', 'og_eng': 'act', 'og_dma': 4,
              'out_bufs': 2, 'kpot_m3': True, 'debug': False}, **(cfg or {})}
    nc = bass.Bass("TRN2")
    lhs = nc.dram_tensor("lhs", [48, N_NODE], U16, kind="ExternalInput")
    rhs = nc.dram_tensor("rhs", [48, KV], U16, kind="ExternalInput")
    # kvcat rows 0:64 = v dims, 64:128 = k dims; chunk-major free axis
    kvcat = nc.dram_tensor("kvcat", [128, N_NODE], FP32, kind="ExternalInput")
    kvl = nc.dram_tensor("kvl", [128, N_NODE], U16, kind="ExternalInput")
    qs = nc.dram_tensor("qs", [128, N_NODE], U16, kind="ExternalInput")
    q8z = nc.dram_tensor("q8z", [64, NCHUNK * 256], U8, kind="ExternalInput")
    # out_g[p, g*512 + (c%8)*64 + d] = out[c*128 + p, d],  g = c//8
    out_g = nc.dram_tensor("out_g", [128, 1024], FP32, kind="ExternalOutput")
    rs_out = nc.dram_tensor("rs_out", [128, NCHUNK], FP32, kind="ExternalOutput")
    dbg = {}
    if cfg['debug']:
        for nm, shape, dt in (("d_SC0", [128, 608], FP32),
                              ("d_SCl0", [128, 608], FP16),
                              ("d_KC", [128, KVE], FP32),
                              ("d_KS", [128, KVE], FP32),
                              ("d_KRTh", [128, KVE], FP16),
                              ("d_KITh", [128, KVE], FP16),
                              ("d_KRTl8", [64, 2 * KVE], U8),
                              ("d_AB", [128, 1024], FP32),
                              ("d_nL", [128, KV], FP32),
                              ("d_sml", [128, 3], FP32),
                              ("d_E", [128, KV], FP32),
                              ("d_G", [128, 384], FP16),
                              ("d_nH", [128, 384], FP16),
                              ("d_VRsb", [128, 192], FP16),
                              ("d_VIsb", [128, 192], FP16)):
            dbg[nm] = nc.dram_tensor(nm, shape, dt, kind="ExternalOutput")

    Sin = mybir.ActivationFunctionType.Sin
    Exp = mybir.ActivationFunctionType.Exp
    Copy = mybir.ActivationFunctionType.Copy
    mult = mybir.AluOpType.mult
    sub = mybir.AluOpType.subtract
    amin = mybir.AluOpType.min
    amax = mybir.AluOpType.max
    DR = mybir.MatmulPerfMode.DoubleRow

    GB = cfg['gh_bufs']
    NH = N_NODE // 2
    DC = cfg['ght_dve_cols']

    with tile.TileContext(nc) as tc:
        with tc.tile_pool(name="consts", bufs=1) as consts, \
             tc.tile_pool(name="work", bufs=cfg['work_bufs']) as work, \
             tc.tile_pool(name="scl", bufs=cfg['scl_bufs']) as sclp, \
             tc.tile_pool(name="ghts", bufs=cfg['ght_bufs']) as ghts, \
             tc.tile_pool(name="small", bufs=4) as small:

            t_lhs = consts.tile([48, N_NODE], U16, tag="lhs")
            t_rhs = consts.tile([48, KV], U16, tag="rhs")
            t_kv = consts.tile([128, N_NODE], FP32, tag="kv")
            t_kvl = consts.tile([128, N_NODE], U16, tag="kvl")
            t_qs = consts.tile([128, N_NODE], U16, tag="qs")
            t_q8z = consts.tile([64, NCHUNK * 256], U8, tag="q8z")
            # phase weights first so phase matmuls start ASAP
            nc.sync.dma_start(t_lhs[0:16, :], lhs[0:16, :])
            nc.sync.dma_start(t_rhs, rhs[:])
            nc.sync.dma_start(t_lhs[16:48, :], lhs[16:48, :])
            NQ = N_NODE // 4
            for i in range(4):
                s = slice(i * NQ, (i + 1) * NQ)
                nc.sync.dma_start(t_kv[:, s], kvcat[:, s])
                nc.sync.dma_start(t_kvl[:, s], kvl[:, s])
            nc.sync.dma_start(t_qs, qs[:])
            nc.sync.dma_start(t_q8z, q8z[:])

            lb = t_lhs[0:48, :].bitcast(FP16)
            rb = t_rhs[0:48, :].bitcast(FP16)

            # kv bf16-hi view: fp32 high half-words (odd u16 index)
            kv_bf = t_kv[:].bitcast(BF16).rearrange(
                "p (c d two) -> p c d two", c=NCHUNK, two=2)
            kvl_r = t_kvl[:].bitcast(FP16).rearrange(
                "p (c d) -> p c d", c=NCHUNK)
            qs_r = t_qs[:].bitcast(BF16).rearrange(
                "p (c n) -> p c n", c=NCHUNK)
            q8_r = t_q8z[:].bitcast(FP8E4).rearrange(
                "p (c two n) -> p c two n", c=NCHUNK, two=2)

            from concourse.masks import make_identity
            t_ident = consts.tile([128, 128], FP16, tag="ident")
            make_identity(nc, t_ident)
            # PE warmup: burn the p-state ramp during the input DMAs
            with tc.tile_pool(name="ps_warm", bufs=1, space="PSUM") as ps_w:
                pw = ps_w.tile([128, 128], FP16, tag="warm")
                for _ in range(14):
                    nc.tensor.transpose(pw, t_ident, t_ident)

            # S|C fp32 slabs: [-S at 0:304 | C at 304:608]
            t_SC = [consts.tile([128, 608], FP32, tag=f"SC{c}",
                                name=f"SCt{c}") for c in range(NCHUNK)]
            for c in range(NCHUNK):
                pad = t_SC[c][:].rearrange(
                    "p (b x) -> p b x", b=2)[:, :, KV:304]
                nc.vector.memset(pad, 0.0)
            # fp16 residual slabs [Sl | Cl] (ring)
            t_SCl = [sclp.tile([128, 608], FP16, tag="SCl",
                               name=f"SCl{i}") for i in range(cfg['scl_bufs'])]
            for i in range(cfg['scl_bufs']):
                pad = t_SCl[i][:].rearrange(
                    "p (b x) -> p b x", b=2)[:, :, KV:304]
                nc.vector.memset(pad, 0.0)

            # G / -H rings [128, 384] fp16, pads zeroed once
            t_G = [consts.tile([128, 384], FP16, tag=f"G{i}", name=f"Gt{i}")
                   for i in range(GB)]
            t_nH = [consts.tile([128, 384], FP16, tag=f"H{i}", name=f"Ht{i}")
                    for i in range(GB)]
            for i in range(GB):
                nc.vector.memset(t_G[i][:, KV:384], 0.0)
                nc.vector.memset(t_nH[i][:, KV:384], 0.0)

            t_dummy = small.tile([128, 1], FP32, tag="dummy")
            t_rs = consts.tile([128, NCHUNK], FP32, tag="rs_all")

            # ---------------- phase 1: trig + k_pot/v_pot accumulation
            # per chunk: 2 phase mms -> Sin -> TTR residual split (DVE) and
            # 6 kpot mms; kpot(c-1) emitted after phase(c) so the PE never
            # head-of-line blocks on the Sin chain.
            def sc_views(c):
                sc = t_SC[c]
                ch = sc[:].bitcast(BF16).rearrange(
                    "p (b x two) -> p b x two", b=2, two=2)[:, 1, 0:KVE, 1]
                sh = sc[:].bitcast(BF16).rearrange(
                    "p (b x two) -> p b x two", b=2, two=2)[:, 0, 0:KVE, 1]
                return sh, ch

            def emit_kpot_main(c):
                shv, chv = sc_views(c)
                kvh = kv_bf[:, c, :, 1]
                # main: kvH^T Ch / kvH^T (-S)h
                nc.tensor.matmul(psKC, kvh, chv,
                                 start=(c == 0), stop=False)
                nc.tensor.matmul(psKS, kvh, shv,
                                 start=(c == 0), stop=False)
                if cfg['kpot_m3']:
                    # m3: kvl^T Ch / kvl^T Sh
                    nc.tensor.matmul(psKC, kvl_r[:, c, :], chv,
                                     start=False, stop=False)
                    nc.tensor.matmul(psKS, kvl_r[:, c, :], shv,
                                     start=False, stop=False)

            def emit_kpot_m2(c):
                # m2: kvH^T Cl / kvH^T Sl (shifted: SCl comes off the
                # Sin->TTR chain, so give it 2 chunks of slack)
                sl_scl = t_SCl[c % cfg['scl_bufs']]
                kvh = kv_bf[:, c, :, 1]
                nc.tensor.matmul(psKC, kvh, sl_scl[:, 304:304 + KVE],
                                 start=False, stop=(c == NCHUNK - 1))
                nc.tensor.matmul(psKS, kvh, sl_scl[:, 0:KVE],
                                 start=False, stop=(c == NCHUNK - 1))

            with tc.tile_pool(name="ps_ph", bufs=cfg['ph_bufs'],
                              space="PSUM") as ps_ph, \
                 tc.tile_pool(name="ps_kc", bufs=1, space="PSUM") as ps_kc:
                psKC = ps_kc.tile([128, KVE], FP32, tag="KC")
                psKS = ps_kc.tile([128, KVE], FP32, tag="KS")
                for c in range(NCHUNK):
                    sl = slice(c * 128, (c + 1) * 128)
                    pSC = ps_ph.tile([128, 1024], FP32, tag="pSC")
                    # bank1 (C): rows 0:13; bank0 (-S): rows 32:43
                    nc.tensor.matmul(pSC[:, 512:512 + KV], lb[0:13, sl],
                                     rb[0:13, :], start=True, stop=True,
                                     tile_position=(0, 0))
                    nc.tensor.matmul(pSC[:, 0:KV], lb[32:43, sl],
                                     rb[32:43, :], start=True, stop=True,
                                     tile_position=(32, 0))
                    pin = pSC[:].rearrange("p (b x) -> p b x", b=2)[:, :, 0:KV]
                    sout = t_SC[c][:].rearrange(
                        "p (b x) -> p b x", b=2)[:, :, 0:KV]
                    nc.scalar.activation(sout, pin, Sin, scale=-TWOPI)
                    # residual split: [Sl|Cl] = SC - bf16trunc(SC), fp16 out
                    scl = t_SCl[c % cfg['scl_bufs']]
                    sc_b = t_SC[c][:].bitcast(BF16).rearrange(
                        "p (b x two) -> p b x two", b=2, two=2)[:, :, 0:KV, 1]
                    nc.vector.tensor_tensor(
                        scl[:].rearrange("p (b x) -> p b x", b=2)[:, :, 0:KV],
                        sout, sc_b, sub)
                    if c >= 1:
                        emit_kpot_main(c - 1)
                    if c >= 2:
                        emit_kpot_m2(c - 2)
                emit_kpot_main(NCHUNK - 1)
                emit_kpot_m2(NCHUNK - 2)
                emit_kpot_m2(NCHUNK - 1)

                # ---- k_pot / v_pot staging (one-time) --------------------
                # KRTh/KITh' fp16 duplicated on both 64-partition halves
                t_KRTh = consts.tile([128, KVE], FP16, tag="KRTh")
                t_KITh = consts.tile([128, KVE], FP16, tag="KITh")
                nc.scalar.activation(t_KITh[0:64, :], psKS[64:128, :], Copy)
                nc.scalar.activation(t_KITh[64:128, :], psKS[64:128, :], Copy)
                nc.vector.tensor_copy(t_KRTh[0:64, :], psKC[64:128, :])
                nc.vector.tensor_copy(t_KRTh[64:128, :], psKC[64:128, :])
                # fp8 residuals (x16) for the DoubleRow corrections
                t_KRTl8 = consts.tile([64, 2 * KVE], FP8E4, tag="KRTl8")
                t_KITl8 = consts.tile([64, 2 * KVE], FP8E4, tag="KITl8")
                nc.vector.memset(t_KRTl8[:, KVE:], 0.0)
                nc.vector.memset(t_KITl8[:, KVE:], 0.0)
                t_Ltmp = consts.tile([64, 2 * KVE], FP32, tag="Ltmp")
                nc.vector.tensor_tensor(t_Ltmp[:, 0:KVE], psKC[64:128, :],
                                        t_KRTh[0:64, :], sub)
                nc.vector.tensor_tensor(t_Ltmp[:, KVE:], psKS[64:128, :],
                                        t_KITh[0:64, :], sub)
                nc.vector.tensor_scalar(t_KRTl8[:, 0:KVE], t_Ltmp[:, 0:KVE],
                                        16.0, None, mult)
                nc.vector.tensor_scalar(t_KITl8[:, 0:KVE], t_Ltmp[:, KVE:],
                                        16.0, None, mult)
                # v_pot: VR = psKC[0:64], VI = -psKS[0:64]
                t_VRT = consts.tile([64, 384], FP16, tag="VRT")
                t_VIT = consts.tile([64, 384], FP16, tag="VIT")
                nc.vector.memset(t_VRT[:, KV:384], 0.0)
                nc.vector.memset(t_VIT[:, KV:384], 0.0)
                nc.scalar.activation(t_VRT[:, 0:KV], psKC[0:64, 0:KV], Copy)
                nc.scalar.activation(t_VIT[:, 0:KV], psKS[0:64, 0:KV], Copy,
                                     scale=-1.0)
                if cfg['debug']:
                    t_dKC = consts.tile([128, KVE], FP32, tag="dKC")
                    t_dKS = consts.tile([128, KVE], FP32, tag="dKS")
                    nc.vector.tensor_copy(t_dKC, psKC[:])
                    nc.vector.tensor_copy(t_dKS, psKS[:])
                    nc.sync.dma_start(dbg["d_KC"][:], t_dKC[:])
                    nc.sync.dma_start(dbg["d_KS"][:], t_dKS[:])

            t_VRsb = consts.tile([128, 192], FP16, tag="VRsb")
            t_VIsb = consts.tile([128, 192], FP16, tag="VIsb")

            ps_ab = tc.alloc_tile_pool(name="ps_ab", bufs=cfg['ab_bufs'],
                                       space="PSUM")
            ps_ght = tc.alloc_tile_pool(name="ps_ght", bufs=cfg['ght_bufs'],
                                        space="PSUM")
            ps_out = tc.alloc_tile_pool(name="ps_out", bufs=cfg['out_bufs'],
                                        space="PSUM")

            # ---------------- phase 2: logits, softmax, inverse transform
            psO = [None] * 2
            t_og_of = [None] * 2
            E_of = [None] * NCHUNK

            def emit_ab_main(c):
                if c % 8 == 0:
                    psO[c // 8] = ps_out.tile([128, 512], FP32, tag="O",
                                              name=f"Ot{c//8}")
                psAB = ps_ab.tile([128, 1024], FP32, tag="AB",
                                  name=f"psAB{c}")
                # bank0 = q^T KIT' (= -B), bank1 = q^T KRT (= A)
                nc.tensor.matmul(psAB[:, 0:KVE], qs_r[:, c, :], t_KITh,
                                 start=True, stop=(cfg['logits_corr'] != 'dr'))
                nc.tensor.matmul(psAB[:, 512:512 + KVE], qs_r[:, c, :],
                                 t_KRTh,
                                 start=True, stop=(cfg['logits_corr'] != 'dr'))
                return psAB

            def emit_ab_corr(c, psAB):
                if cfg['logits_corr'] == 'dr':
                    q8c = q8_r[:, c, :, :]
                    nc.tensor.matmul(
                        psAB[:, 0:KVE], q8c,
                        t_KITl8[:].rearrange("p (two x) -> p two x", two=2),
                        start=False, stop=True, perf_mode=DR,
                        skip_group_check=True)
                    nc.tensor.matmul(
                        psAB[:, 512:512 + KVE], q8c,
                        t_KRTl8[:].rearrange("p (two x) -> p two x", two=2),
                        start=False, stop=True, perf_mode=DR,
                        skip_group_check=True)

            def emit_softmax(c, psAB):
                Cc = t_SC[c][:, 304:304 + KV]
                nSc = t_SC[c][:, 0:KV]
                scp = t_SC[c][:].rearrange("p (b x) -> p b x", b=2)[:, :, 0:KV]
                abp = psAB[:].rearrange("p (b x) -> p b x", b=2)[:, :, 0:KV]
                # T12 = [(-S)(-B) | C*A] = [SB | CA] in one DVE pass
                T12 = work.tile([128, 608], FP32, tag="T12", name=f"T12_{c}")
                t12p = T12[:].rearrange("p (b x) -> p b x", b=2)[:, :, 0:KV]
                nc.vector.tensor_tensor(t12p, scp, abp, mult)
                # L = CA - SB (DVE; keeps the T12->L->max chain one-queue)
                Lt = work.tile([128, KV], FP32, tag="nL", name=f"nL{c}")
                nc.vector.tensor_tensor(Lt, T12[:, 304:304 + KV],
                                        T12[:, 0:KV], sub)
                negmx = small.tile([128, 1], FP32, tag="negmx")
                nc.vector.tensor_reduce(negmx, Lt, mybir.AxisListType.X,
                                        amax, negate=True)
                # E = exp(L - max L); rs lands in rs_all - the softmax
                # normalization (divide by rs) happens on the host
                E = work.tile([128, KV], FP32, tag="E", name=f"E{c}")
                nc.scalar.activation(E, Lt, Exp, bias=negmx[:, 0:1],
                                     scale=1.0, accum_out=t_rs[:, c:c + 1])
                E_of[c] = E
                if cfg['debug'] and c == 0:
                    nc.sync.dma_start(dbg["d_SC0"][:], t_SC[0][:])
                    nc.sync.dma_start(dbg["d_SCl0"][:], t_SCl[0][:])
                    nc.sync.dma_start(dbg["d_KRTh"][:], t_KRTh[:])
                    nc.sync.dma_start(dbg["d_KITh"][:], t_KITh[:])
                    nc.sync.dma_start(dbg["d_KRTl8"][:],
                                      t_KRTl8[:].bitcast(U8))
                    t_dAB = consts.tile([128, 1024], FP32, tag="dAB")
                    nc.vector.tensor_copy(t_dAB, psAB[:])
                    nc.sync.dma_start(dbg["d_AB"][:], t_dAB[:])
                    nc.sync.dma_start(dbg["d_nL"][:], Lt[:])
                    nc.sync.dma_start(dbg["d_sml"][:, 0:1], negmx[:])
                    nc.sync.dma_start(dbg["d_sml"][:, 1:2], rs[:])
                    nc.sync.dma_start(dbg["d_sml"][:, 2:3], rrs[:])
                    nc.sync.dma_start(dbg["d_E"][:], E[:])


            def emit_gh(c):
                # G = E*C (DVE), -H = E*(-S) (Pool); 1/rs folds into og.
                # Emitted one chunk late so the DVE/Pool queues never wait
                # on the exp of the chunk they just fed.
                E = E_of[c]
                Cc = t_SC[c][:, 304:304 + KV]
                nSc = t_SC[c][:, 0:KV]
                nc.gpsimd.tensor_tensor(t_G[c % GB][:, 0:KV], E, Cc, mult)
                nc.gpsimd.tensor_tensor(t_nH[c % GB][:, 0:KV], E, nSc, mult)

            def emit_tf(c):
                G = t_G[c % GB]
                nH = t_nH[c % GB]
                pst = ps_ght.tile([128, 768], FP16, tag="ght", name=f"pst{c}")
                for j in range(3):
                    jsl = slice(j * 128, (j + 1) * 128)
                    nc.tensor.transpose(pst[:, j * 128:(j + 1) * 128],
                                        G[:, jsl], t_ident)
                    nc.tensor.transpose(
                        pst[:, 384 + j * 128:384 + (j + 1) * 128],
                        nH[:, jsl], t_ident)
                GHT = ghts.tile([128, 768], FP16, tag="ghts", name=f"GHT{c}")
                if DC > 0:
                    nc.vector.tensor_copy(GHT[:, 0:DC], pst[:, 0:DC])
                if DC < 768:
                    nc.scalar.activation(GHT[:, DC:768], pst[:, DC:768], Copy)
                og = psO[c // 8][:, (c % 8) * 64:(c % 8 + 1) * 64]
                for j in range(3):
                    w = 128 if j < 2 else KV - 256
                    nc.tensor.matmul(og, GHT[0:w, j * 128:(j + 1) * 128],
                                     t_VRsb[0:w, j * 64:(j + 1) * 64],
                                     start=(j == 0), stop=False)
                    nc.tensor.matmul(
                        og, GHT[0:w, 384 + j * 128:384 + (j + 1) * 128],
                        t_VIsb[0:w, j * 64:(j + 1) * 64],
                        start=False, stop=(j == 2))
                # PSUM cannot source a DMA: stage og in SBUF. Plain copy
                # (host divides by rs), batched per 2 chunks at odd c.
                g = c // 8
                if t_og_of[g] is None:
                    t_og_of[g] = work.tile([128, 512], FP32, tag="og",
                                           name=f"og{g}")
                t_og = t_og_of[g]
                if c % 2 == 1:
                    osl = slice((c % 8 - 1) * 64, (c % 8 + 1) * 64)
                    ogs = psO[g][:, osl]
                    if cfg['og_eng'] == 'act':
                        nc.scalar.activation(t_og[:, osl], ogs, Copy)
                    else:
                        nc.vector.tensor_copy(t_og[:, osl], ogs)
                if cfg['debug'] and c == 0:
                    nc.sync.dma_start(dbg["d_VRsb"][:], t_VRsb[:])
                    nc.sync.dma_start(dbg["d_VIsb"][:], t_VIsb[:])
                q = cfg['og_dma']
                step = 8 // q
                if (c % 8) % step == step - 1:
                    i0 = ((c % 8) // step) * 64 * step
                    i1 = i0 + 64 * step
                    nc.sync.dma_start(out_g[:, g * 512 + i0:g * 512 + i1],
                                      t_og[:, i0:i1])

            vr_emitted = False

            def emit_vr_prep():
                pvr = ps_ght.tile([128, 768], FP16, tag="ght", name="pvr")
                for j in range(3):
                    jsl = slice(j * 128, (j + 1) * 128)
                    nc.tensor.transpose(pvr[:, j * 64:(j + 1) * 64],
                                        t_VRT[:, jsl], t_ident[0:64, 0:64])
                    nc.tensor.transpose(
                        pvr[:, 192 + j * 64:192 + (j + 1) * 64],
                        t_VIT[:, jsl], t_ident[0:64, 0:64])
                nc.vector.tensor_copy(t_VRsb, pvr[:, 0:192])
                nc.vector.tensor_copy(t_VIsb, pvr[:, 192:384])

            SH = cfg['shift']
            psAB_of = [None] * NCHUNK
            for c in range(NCHUNK):
                psAB_of[c] = emit_ab_main(c)
                if c >= 1:
                    emit_ab_corr(c - 1, psAB_of[c - 1])
                if not vr_emitted:
                    emit_vr_prep()
                    vr_emitted = True
                if c >= 1:
                    emit_softmax(c - 1, psAB_of[c - 1])
                if c >= 2:
                    emit_gh(c - 2)
                if c >= 2 + SH:
                    emit_tf(c - 2 - SH)
            emit_ab_corr(NCHUNK - 1, psAB_of[NCHUNK - 1])
            emit_softmax(NCHUNK - 1, psAB_of[NCHUNK - 1])
            emit_gh(NCHUNK - 2)
            emit_gh(NCHUNK - 1)
            for c in range(NCHUNK - 2 - SH, NCHUNK):
                emit_tf(c)
            nc.sync.dma_start(rs_out[:], t_rs[:])

            ps_out.release()
            ps_ght.release()
            ps_ab.release()
    return nc


_NC_CACHE = {}


def _get_nc(cfg=None):
    key = tuple(sorted((cfg or {}).items()))
    if key not in _NC_CACHE:
        _install_bir_patch()
        _NC_CACHE[key] = _build_nc(cfg)
    return _NC_CACHE[key]


# ------------------------------------------------------------- host wrapper
def _kvecs(nk):
    kx = np.arange(0, nk[0] + 1)
    ky = np.arange(-nk[1], nk[1] + 1)
    kz = np.arange(-nk[2], nk[2] + 1)
    KX, KY, KZ = np.meshgrid(kx, ky, kz, indexing="ij")
    return np.stack([KX, KY, KZ], axis=-1).reshape(-1, 3).astype(np.float64)


def _bf16_trunc(x):
    """bf16 truncation (high 16 bits) of fp32, as fp32."""
    xv = np.ascontiguousarray(x, dtype=np.float32).view(np.uint32)
    return (xv & 0xFFFF0000).view(np.float32)


def _prep_core_inputs(q, k, v, r, box):
    f = (r.astype(np.float64) / box[None, :])  # [n,3] in [0,1)

    nk = [max(1, int(b)) for b in (box / DL).astype(np.int64)]
    kvs = _kvecs(nk)
    ksq = TWOPI ** 2 * ((kvs / box[None, :]) ** 2).sum(-1)
    valid = (ksq <= K_SQ_MAX) & (ksq > 0)
    kint = kvs[valid]  # [KV,3] small integers
    assert kint.shape[0] == KV, f"valid k-points {kint.shape[0]} != {KV}"
    kcolT = kint.T  # [3, KV]

    fh = f.astype(np.float16).astype(np.float64)
    fl = (f - fh).astype(np.float16).astype(np.float64)

    lhsb = np.zeros((48, N_NODE), np.float16)
    rhsb = np.zeros((48, KV), np.float16)
    # bank1 (C): accumulate p_h + 1/4 -> round -> -p_h - p_l - 1/4
    lhsb[0:3] = fh.T
    lhsb[3] = 1.0
    lhsb[4] = MAGIC_A
    lhsb[5] = MAGIC_A
    lhsb[6:9] = fh.T
    lhsb[9:12] = fl.T
    lhsb[12] = 1.0
    rhsb[0:3] = kcolT
    rhsb[3] = 0.25
    rhsb[4] = MAGIC_B
    rhsb[5] = -MAGIC_B
    rhsb[6:9] = -kcolT
    rhsb[9:12] = -kcolT
    rhsb[12] = -0.25
    # bank0 (-S): accumulate -p_h -> round -> +p_h + p_l
    lhsb[32:35] = fh.T
    lhsb[35] = MAGIC_A
    lhsb[36] = MAGIC_A
    lhsb[37:40] = fh.T
    lhsb[40:43] = fl.T
    rhsb[32:35] = -kcolT
    rhsb[35] = MAGIC_B
    rhsb[36] = -MAGIC_B
    rhsb[37:40] = kcolT
    rhsb[40:43] = kcolT

    # kv concat: rows 0:64 v, 64:128 k; chunk-major free axis
    kvc = np.concatenate([v, k], axis=1)  # [n,128]
    kvcat = np.ascontiguousarray(
        kvc.reshape(NCHUNK, 128, 128).transpose(1, 0, 2).reshape(128, N_NODE)
    ).astype(np.float32)
    kvl16 = (kvcat - _bf16_trunc(kvcat)).astype(np.float16)

    # q bf16 hi/lo pair, stacked on partitions: rows 0:64 hi, 64:128 lo
    qT = np.ascontiguousarray(q.T).astype(np.float32)  # [64, n]
    qh = qT.astype(ml_dtypes.bfloat16)
    ql = (qT - qh.astype(np.float32)).astype(ml_dtypes.bfloat16)
    qs16 = np.concatenate([qh.view(np.uint16), ql.view(np.uint16)], axis=0)

    # fp8 q/16 with zero slot1 per chunk: [64, c, 2, 128]
    q8 = (qT / 16.0).astype(ml_dtypes.float8_e4m3)
    q8z = np.zeros((64, NCHUNK, 2, 128), ml_dtypes.float8_e4m3)
    q8z[:, :, 0, :] = q8.reshape(64, NCHUNK, 128)

    return {"lhs": lhsb.view(np.uint16), "rhs": rhsb.view(np.uint16),
            "kvcat": kvcat, "kvl": kvl16.view(np.uint16),
            "qs": qs16, "q8z": q8z.reshape(64, -1).view(np.uint8)}


def kernel(q_vector, k_vector, v_vector, positions, cell, batch):
    q_vector = np.asarray(q_vector)
    k_vector = np.asarray(k_vector)
    v_vector = np.asarray(v_vector)
    positions = np.asarray(positions)
    cell = np.asarray(cell)

    n = N_NODE
    boxes = np.diagonal(cell.reshape(-1, 3, 3), axis1=-2, axis2=-1)  # [B,3]

    in_maps = []
    for b in range(B):
        sl = slice(b * n, (b + 1) * n)
        in_maps.append(_prep_core_inputs(
            q_vector[sl], k_vector[sl], v_vector[sl], positions[sl],
            boxes[b].astype(np.float64)))

    nc = _get_nc()
    res = None
    last_err = None
    for _attempt in range(3):
        try:
            res = run_bass_kernel_spmd(nc, in_maps, list(range(B)))
            break
        except Exception as e:  # transient device states
            last_err = e
    if res is None:
        raise last_err

    out = np.empty((B * n, D), np.float32)
    for b in range(B):
        og = res.results[b]["out_g"].reshape(128, 2, 8, D)  # [p, g, c8, d]
        o = og.transpose(1, 2, 0, 3).reshape(n, D)
        rs = res.results[b]["rs_out"].T.reshape(n, 1)  # [c*128+p]
        out[b * n:(b + 1) * n] = o / rs
    return out


if __name__ == "__main__":
    rng = np.random.default_rng(0)
    inputs = {
        "q_vector": rng.standard_normal((B * N_NODE, D), dtype=np.float32),
        "k_vector": rng.standard_normal((B * N_NODE, D), dtype=np.float32),
        "v_vector": rng.standard_normal((B * N_NODE, D), dtype=np.float32),
        "positions": rng.uniform(0, 20, (B * N_NODE, 3)).astype(np.float32),
        "cell": np.tile((np.eye(3, dtype=np.float32) * 20.0)[None], (B, 1, 1)),
        "batch": np.repeat(np.arange(B, dtype=np.int32), N_NODE),
    }
    o = kernel(**inputs)
    print("kernel ran, out", o.shape, o.dtype, float(np.abs(o).max()))


# revision 9
# speedup vs baseline: 1.1482x; 1.0225x over previous
"""Trainium2 Bass kernel for nn_EwaldPotential — v3.

Math per graph (all-real form of the complex reference):
  p[i,k] = (r[i,:]/box) @ kvecs[k,:]      (phase in cycles)
  C = cos(2*pi*p), S = sin(2*pi*p)
  KRT[d,k] = sum_i k[i,d] C[i,k]; KIT from S; VR/VI likewise from v
  A = q @ KRT, B = q @ KIT;  L = C*A - S*B
  P = softmax_k(L);  out = (P*C) @ VR - (P*S) @ VI

v3 design (all matmuls 1 cyc/row, exact-split precision):
 - phase: ONE fp16 11/13-row matmul per bank (f split fp16-hi/fp16-lo,
   magic rounding via the exact product 256*49152 = 1.5*2^23). Bank0
   accumulates frac(-p) so Sin gives -S; bank1 frac(p+1/4) gives C.
 - kpot: kv^T C split as kvH^T Ch + kvH^T Cl + kvl^T Ch (kvH/Ch are
   FREE bf16-truncation bitcast views of the fp32 tensors; kvl host
   fp16 residual; Cl/Sl one fused DVE TTR-sub per chunk, fp16 out,
   running in the otherwise-idle phase-1 DVE window).
 - logits: stationary q as bf16 hi/lo pair stacked on 128 partitions
   (exact to 2^-18) against fp16 KRTh duplicated on both halves; the
   fp16-residual KRTl correction runs as one fp8e4 DoubleRow matmul
   per output (0.5 cyc/row).
 - softmax: T12 = [SB | CA] one DVE pass ((-S)*(-B) = SB); fused DVE
   TTR: negL = SB-CA = -L with min-accum = -max(L); exp(scale=-1,
   bias=negmx) on ACT with rs accum; rrs folded into the Pool STTs
   G = (E*rrs)*C and -H = (E*rrs)*(-S), so the output needs no
   normalization copy: finals accumulate G@VR + (-H)@VI in PSUM and
   out DMAs straight from PSUM.
 - engine balance: ACT: Sin, exp, GHT tail; DVE: SCl sub (phase 1),
   T12, TTR, recip, GHT head; Pool: G/H STTs; PE: everything matmul.
"""
import json
import numpy as np
import ml_dtypes

import concourse.bass as bass
import concourse.tile as tile
from concourse import mybir
from concourse.bass_utils import run_bass_kernel_spmd

# ---------------------------------------------------------------- constants
B = 8
N_NODE = 2048
D = 64
DL = 4.0
TWOPI = 2.0 * np.pi
K_SQ_MAX = (TWOPI / DL) ** 2
KV = 297                    # valid k-point count
KVE = 298                   # even width for matmul outputs
NCHUNK = N_NODE // 128
MAGIC_A = 256.0             # MAGIC_A * MAGIC_B = 1.5 * 2^23 exactly
MAGIC_B = 49152.0

FP32 = mybir.dt.float32
FP32R = mybir.dt.float32r
BF16 = mybir.dt.bfloat16
FP16 = mybir.dt.float16
FP8E4 = mybir.dt.float8e4
U16 = mybir.dt.uint16
U8 = mybir.dt.uint8

# ------------------------------------------------- walrus wait-split patch
_ws_counter = [0]


def _split_waits_json(bir_bytes: bytes) -> bytes:
    d = json.loads(bir_bytes)
    changed = False
    for fn in d.get("functions", []):
        for blk in fn.get("blocks", []):
            out = []
            for ins in blk.get("instructions", []):
                si = ins.get("sync_info")
                ow = (si or {}).get("on_wait") or []
                if len(ow) > 1:
                    changed = True
                    for w in ow[:-1]:
                        _ws_counter[0] += 1
                        out.append({
                            "debug": ins.get("debug", 0),
                            "engine": ins.get("engine"),
                            "ins": [], "outs": [],
                            "name": f"I-wsplit{_ws_counter[0]}",
                            "opcode": "NoOp",
                            "sync_info": {"on_wait": [w], "on_update": []},
                        })
                    si["on_wait"] = [ow[-1]]
                out.append(ins)
            blk["instructions"] = out
    return json.dumps(d).encode() if changed else bir_bytes


def _install_bir_patch():
    import concourse.bass_utils as bu
    import concourse.bass2jax as b2j

    if getattr(b2j.compile_bir_kernel, "_wait_split", False):
        return
    orig = bu.compile_bir_kernel

    def wrapped(bir_json, tmpdir, neff_name="file.neff"):
        return orig(_split_waits_json(bir_json), tmpdir, neff_name)

    wrapped._wait_split = True
    b2j.compile_bir_kernel = wrapped
    bu.compile_bir_kernel = wrapped


# ------------------------------------------------------------ device kernel
def _build_nc(cfg=None):
    cfg = {**{'work_bufs': 4, 'ab_bufs': 2, 'ght_bufs': 2, 'gh_bufs': 4,
              'scl_bufs': 3, 'ph_bufs': 3, 'shift': 2, 'ght_dve_cols': 16,
              'logits_corr': 'dr', 'og_eng': 'act', 'og_dma': 4,
              'out_bufs': 2, 'kpot_m3': True, 'drain': 3, 'warm': 14,
              'debug': False}, **(cfg or {})}
    nc = bass.Bass("TRN2")
    lhs = nc.dram_tensor("lhs", [48, N_NODE + 304], U16, kind="ExternalInput")
    # kvcat rows 0:64 = v dims, 64:128 = k dims; chunk-major free axis
    kvcat = nc.dram_tensor("kvcat", [128, N_NODE], FP32, kind="ExternalInput")
    kvl = nc.dram_tensor("kvl", [128, N_NODE], U16, kind="ExternalInput")
    qs = nc.dram_tensor("qs", [128, N_NODE], U16, kind="ExternalInput")
    q8z = nc.dram_tensor("q8z", [64, NCHUNK * 256], U8, kind="ExternalInput")
    # out_g[p, g*512 + (c%8)*64 + d] = out[c*128 + p, d],  g = c//8
    out_g = nc.dram_tensor("out_g", [128, 1024], FP32, kind="ExternalOutput")
    rs_out = nc.dram_tensor("rs_out", [128, NCHUNK], FP32, kind="ExternalOutput")
    dbg = {}
    if cfg['debug']:
        for nm, shape, dt in (("d_SC0", [128, 608], FP32),
                              ("d_SCl0", [128, 608], FP16),
                              ("d_KC", [128, KVE], FP32),
                              ("d_KS", [128, KVE], FP32),
                              ("d_KRTh", [128, KVE], FP16),
                              ("d_KITh", [128, KVE], FP16),
                              ("d_KRTl8", [64, 2 * KVE], U8),
                              ("d_AB", [128, 1024], FP32),
                              ("d_nL", [128, KV], FP32),
                              ("d_sml", [128, 3], FP32),
                              ("d_E", [128, KV], FP32),
                              ("d_G", [128, 384], FP16),
                              ("d_nH", [128, 384], FP16),
                              ("d_VRsb", [128, 192], FP16),
                              ("d_VIsb", [128, 192], FP16)):
            dbg[nm] = nc.dram_tensor(nm, shape, dt, kind="ExternalOutput")

    Sin = mybir.ActivationFunctionType.Sin
    Exp = mybir.ActivationFunctionType.Exp
    Copy = mybir.ActivationFunctionType.Copy
    mult = mybir.AluOpType.mult
    sub = mybir.AluOpType.subtract
    amin = mybir.AluOpType.min
    amax = mybir.AluOpType.max
    DR = mybir.MatmulPerfMode.DoubleRow

    GB = cfg['gh_bufs']
    NH = N_NODE // 2
    DC = cfg['ght_dve_cols']

    with tile.TileContext(nc) as tc:
        with tc.tile_pool(name="consts", bufs=1) as consts, \
             tc.tile_pool(name="work", bufs=cfg['work_bufs']) as work, \
             tc.tile_pool(name="scl", bufs=cfg['scl_bufs']) as sclp, \
             tc.tile_pool(name="ghts", bufs=cfg['ght_bufs']) as ghts, \
             tc.tile_pool(name="small", bufs=4) as small:

            t_lhs = consts.tile([48, N_NODE + 304], U16, tag="lhs")
            t_kv = consts.tile([128, N_NODE], FP32, tag="kv")
            t_kvl = consts.tile([128, N_NODE], U16, tag="kvl")
            t_qs = consts.tile([128, N_NODE], U16, tag="qs")
            t_q8z = consts.tile([64, NCHUNK * 256], U8, tag="q8z")
            # phase weights first so phase matmuls start ASAP
            nc.sync.dma_start(t_lhs, lhs[:])
            NQ = N_NODE // 4
            for i in range(4):
                s = slice(i * NQ, (i + 1) * NQ)
                nc.sync.dma_start(t_kv[:, s], kvcat[:, s])
                nc.sync.dma_start(t_kvl[:, s], kvl[:, s])
            nc.sync.dma_start(t_qs, qs[:])
            nc.sync.dma_start(t_q8z, q8z[:])

            lb = t_lhs[0:48, 0:N_NODE].bitcast(FP16)
            rb = t_lhs[0:48, N_NODE:N_NODE + KV].bitcast(FP16)

            # kv bf16-hi view: fp32 high half-words (odd u16 index)
            kv_bf = t_kv[:].bitcast(BF16).rearrange(
                "p (c d two) -> p c d two", c=NCHUNK, two=2)
            kvl_r = t_kvl[:].bitcast(FP16).rearrange(
                "p (c d) -> p c d", c=NCHUNK)
            qs_r = t_qs[:].bitcast(BF16).rearrange(
                "p (c n) -> p c n", c=NCHUNK)
            q8_r = t_q8z[:].bitcast(FP8E4).rearrange(
                "p (c two n) -> p c two n", c=NCHUNK, two=2)

            from concourse.masks import make_identity
            t_ident = consts.tile([128, 128], FP16, tag="ident")
            make_identity(nc, t_ident)
            # PE warmup: burn the p-state ramp during the input DMAs
            with tc.tile_pool(name="ps_warm", bufs=1, space="PSUM") as ps_w:
                pw = ps_w.tile([128, 128], FP16, tag="warm")
                for _ in range(cfg['warm']):
                    nc.tensor.transpose(pw, t_ident, t_ident)

            # S|C fp32 slabs: [-S at 0:304 | C at 304:608]
            t_SC = [consts.tile([128, 608], FP32, tag=f"SC{c}",
                                name=f"SCt{c}") for c in range(NCHUNK)]
            for c in range(NCHUNK):
                pad = t_SC[c][:].rearrange(
                    "p (b x) -> p b x", b=2)[:, :, KV:304]
                nc.vector.memset(pad, 0.0)
            # fp16 residual slabs [Sl | Cl] (ring)
            t_SCl = [sclp.tile([128, 608], FP16, tag="SCl",
                               name=f"SCl{i}") for i in range(cfg['scl_bufs'])]
            for i in range(cfg['scl_bufs']):
                pad = t_SCl[i][:].rearrange(
                    "p (b x) -> p b x", b=2)[:, :, KV:304]
                nc.vector.memset(pad, 0.0)

            # G / -H rings [128, 384] fp16, pads zeroed once
            t_G = [consts.tile([128, 384], FP16, tag=f"G{i}", name=f"Gt{i}")
                   for i in range(GB)]
            t_nH = [consts.tile([128, 384], FP16, tag=f"H{i}", name=f"Ht{i}")
                    for i in range(GB)]
            for i in range(GB):
                nc.vector.memset(t_G[i][:, KV:384], 0.0)
                nc.vector.memset(t_nH[i][:, KV:384], 0.0)

            t_dummy = small.tile([128, 1], FP32, tag="dummy")
            t_rs = consts.tile([128, NCHUNK], FP32, tag="rs_all")

            # ---------------- phase 1: trig + k_pot/v_pot accumulation
            # per chunk: 2 phase mms -> Sin -> TTR residual split (DVE) and
            # 6 kpot mms; kpot(c-1) emitted after phase(c) so the PE never
            # head-of-line blocks on the Sin chain.
            def sc_views(c):
                sc = t_SC[c]
                ch = sc[:].bitcast(BF16).rearrange(
                    "p (b x two) -> p b x two", b=2, two=2)[:, 1, 0:KVE, 1]
                sh = sc[:].bitcast(BF16).rearrange(
                    "p (b x two) -> p b x two", b=2, two=2)[:, 0, 0:KVE, 1]
                return sh, ch

            def emit_kpot_main(c):
                shv, chv = sc_views(c)
                kvh = kv_bf[:, c, :, 1]
                # main: kvH^T Ch / kvH^T (-S)h
                nc.tensor.matmul(psKC, kvh, chv,
                                 start=(c == 0), stop=False)
                nc.tensor.matmul(psKS, kvh, shv,
                                 start=(c == 0), stop=False)
                if cfg['kpot_m3']:
                    # m3: kvl^T Ch / kvl^T Sh
                    nc.tensor.matmul(psKC, kvl_r[:, c, :], chv,
                                     start=False, stop=False)
                    nc.tensor.matmul(psKS, kvl_r[:, c, :], shv,
                                     start=False, stop=False)

            def emit_kpot_m2(c):
                # m2: kvH^T Cl / kvH^T Sl (shifted: SCl comes off the
                # Sin->TTR chain, so give it 2 chunks of slack)
                sl_scl = t_SCl[c % cfg['scl_bufs']]
                kvh = kv_bf[:, c, :, 1]
                nc.tensor.matmul(psKC, kvh, sl_scl[:, 304:304 + KVE],
                                 start=False, stop=(c == NCHUNK - 1))
                nc.tensor.matmul(psKS, kvh, sl_scl[:, 0:KVE],
                                 start=False, stop=(c == NCHUNK - 1))

            with tc.tile_pool(name="ps_ph", bufs=cfg['ph_bufs'],
                              space="PSUM") as ps_ph, \
                 tc.tile_pool(name="ps_kc", bufs=1, space="PSUM") as ps_kc:
                psKC = ps_kc.tile([128, KVE], FP32, tag="KC")
                psKS = ps_kc.tile([128, KVE], FP32, tag="KS")
                for c in range(NCHUNK):
                    sl = slice(c * 128, (c + 1) * 128)
                    pSC = ps_ph.tile([128, 1024], FP32, tag="pSC")
                    # bank1 (C): rows 0:13; bank0 (-S): rows 32:43
                    nc.tensor.matmul(pSC[:, 512:512 + KV], lb[0:13, sl],
                                     rb[0:13, :], start=True, stop=True,
                                     tile_position=(0, 0))
                    nc.tensor.matmul(pSC[:, 0:KV], lb[32:43, sl],
                                     rb[32:43, :], start=True, stop=True,
                                     tile_position=(32, 0))
                    pin = pSC[:].rearrange("p (b x) -> p b x", b=2)[:, :, 0:KV]
                    sout = t_SC[c][:].rearrange(
                        "p (b x) -> p b x", b=2)[:, :, 0:KV]
                    nc.scalar.activation(sout, pin, Sin, scale=-TWOPI)
                    # residual split: [Sl|Cl] = SC - bf16trunc(SC), fp16 out
                    scl = t_SCl[c % cfg['scl_bufs']]
                    sc_b = t_SC[c][:].bitcast(BF16).rearrange(
                        "p (b x two) -> p b x two", b=2, two=2)[:, :, 0:KV, 1]
                    nc.vector.tensor_tensor(
                        scl[:].rearrange("p (b x) -> p b x", b=2)[:, :, 0:KV],
                        sout, sc_b, sub)
                    if c >= 1:
                        emit_kpot_main(c - 1)
                    if c >= 2:
                        emit_kpot_m2(c - 2)
                emit_kpot_main(NCHUNK - 1)
                emit_kpot_m2(NCHUNK - 2)
                emit_kpot_m2(NCHUNK - 1)

                # ---- k_pot / v_pot staging (one-time) --------------------
                # KRTh/KITh' fp16 duplicated on both 64-partition halves
                t_KRTh = consts.tile([128, KVE], FP16, tag="KRTh")
                t_KITh = consts.tile([128, KVE], FP16, tag="KITh")
                nc.scalar.activation(t_KITh[0:64, :], psKS[64:128, :], Copy)
                nc.scalar.activation(t_KITh[64:128, :], psKS[64:128, :], Copy)
                nc.vector.tensor_copy(t_KRTh[0:64, :], psKC[64:128, :])
                nc.vector.tensor_copy(t_KRTh[64:128, :], psKC[64:128, :])
                # fp8 residuals (x16) for the DoubleRow corrections
                t_KRTl8 = consts.tile([64, 2 * KVE], FP8E4, tag="KRTl8")
                t_KITl8 = consts.tile([64, 2 * KVE], FP8E4, tag="KITl8")
                nc.vector.memset(t_KRTl8[:, KVE:], 0.0)
                nc.vector.memset(t_KITl8[:, KVE:], 0.0)
                t_Ltmp = consts.tile([64, 2 * KVE], FP32, tag="Ltmp")
                nc.vector.tensor_tensor(t_Ltmp[:, 0:KVE], psKC[64:128, :],
                                        t_KRTh[0:64, :], sub)
                nc.vector.tensor_tensor(t_Ltmp[:, KVE:], psKS[64:128, :],
                                        t_KITh[0:64, :], sub)
                nc.vector.tensor_scalar(t_KRTl8[:, 0:KVE], t_Ltmp[:, 0:KVE],
                                        16.0, None, mult)
                nc.vector.tensor_scalar(t_KITl8[:, 0:KVE], t_Ltmp[:, KVE:],
                                        16.0, None, mult)
                # v_pot: VR = psKC[0:64], VI = -psKS[0:64]
                t_VRT = consts.tile([64, 384], FP16, tag="VRT")
                t_VIT = consts.tile([64, 384], FP16, tag="VIT")
                nc.vector.memset(t_VRT[:, KV:384], 0.0)
                nc.vector.memset(t_VIT[:, KV:384], 0.0)
                nc.scalar.activation(t_VRT[:, 0:KV], psKC[0:64, 0:KV], Copy)
                nc.scalar.activation(t_VIT[:, 0:KV], psKS[0:64, 0:KV], Copy,
                                     scale=-1.0)
                if cfg['debug']:
                    t_dKC = consts.tile([128, KVE], FP32, tag="dKC")
                    t_dKS = consts.tile([128, KVE], FP32, tag="dKS")
                    nc.vector.tensor_copy(t_dKC, psKC[:])
                    nc.vector.tensor_copy(t_dKS, psKS[:])
                    nc.sync.dma_start(dbg["d_KC"][:], t_dKC[:])
                    nc.sync.dma_start(dbg["d_KS"][:], t_dKS[:])

            t_VRsb = consts.tile([128, 192], FP16, tag="VRsb")
            t_VIsb = consts.tile([128, 192], FP16, tag="VIsb")

            ps_ab = tc.alloc_tile_pool(name="ps_ab", bufs=cfg['ab_bufs'],
                                       space="PSUM")
            ps_ght = tc.alloc_tile_pool(name="ps_ght", bufs=cfg['ght_bufs'],
                                        space="PSUM")
            ps_out = tc.alloc_tile_pool(name="ps_out", bufs=cfg['out_bufs'],
                                        space="PSUM")

            # ---------------- phase 2: logits, softmax, inverse transform
            psO = [None] * 2
            t_og_of = [None] * 2
            E_of = [None] * NCHUNK

            def emit_ab_main(c):
                if c % 8 == 0:
                    psO[c // 8] = ps_out.tile([128, 512], FP32, tag="O",
                                              name=f"Ot{c//8}")
                psAB = ps_ab.tile([128, 1024], FP32, tag="AB",
                                  name=f"psAB{c}")
                # bank0 = q^T KIT' (= -B), bank1 = q^T KRT (= A)
                nc.tensor.matmul(psAB[:, 0:KVE], qs_r[:, c, :], t_KITh,
                                 start=True, stop=(cfg['logits_corr'] != 'dr'))
                nc.tensor.matmul(psAB[:, 512:512 + KVE], qs_r[:, c, :],
                                 t_KRTh,
                                 start=True, stop=(cfg['logits_corr'] != 'dr'))
                return psAB

            def emit_ab_corr(c, psAB):
                if cfg['logits_corr'] == 'dr':
                    q8c = q8_r[:, c, :, :]
                    nc.tensor.matmul(
                        psAB[:, 0:KVE], q8c,
                        t_KITl8[:].rearrange("p (two x) -> p two x", two=2),
                        start=False, stop=True, perf_mode=DR,
                        skip_group_check=True)
                    nc.tensor.matmul(
                        psAB[:, 512:512 + KVE], q8c,
                        t_KRTl8[:].rearrange("p (two x) -> p two x", two=2),
                        start=False, stop=True, perf_mode=DR,
                        skip_group_check=True)

            def emit_softmax(c, psAB):
                Cc = t_SC[c][:, 304:304 + KV]
                nSc = t_SC[c][:, 0:KV]
                scp = t_SC[c][:].rearrange("p (b x) -> p b x", b=2)[:, :, 0:KV]
                abp = psAB[:].rearrange("p (b x) -> p b x", b=2)[:, :, 0:KV]
                # T12 = [(-S)(-B) | C*A] = [SB | CA] in one DVE pass
                T12 = work.tile([128, 608], FP32, tag="T12", name=f"T12_{c}")
                t12p = T12[:].rearrange("p (b x) -> p b x", b=2)[:, :, 0:KV]
                nc.vector.tensor_tensor(t12p, scp, abp, mult)
                # L = CA - SB (DVE; keeps the T12->L->max chain one-queue)
                Lt = work.tile([128, KV], FP32, tag="nL", name=f"nL{c}")
                nc.vector.tensor_tensor(Lt, T12[:, 304:304 + KV],
                                        T12[:, 0:KV], sub)
                negmx = small.tile([128, 1], FP32, tag="negmx")
                nc.vector.tensor_reduce(negmx, Lt, mybir.AxisListType.X,
                                        amax, negate=True)
                # E = exp(L - max L); rs lands in rs_all - the softmax
                # normalization (divide by rs) happens on the host
                E = work.tile([128, KV], FP32, tag="E", name=f"E{c}")
                nc.scalar.activation(E, Lt, Exp, bias=negmx[:, 0:1],
                                     scale=1.0, accum_out=t_rs[:, c:c + 1])
                E_of[c] = E
                if cfg['debug'] and c == 0:
                    nc.sync.dma_start(dbg["d_SC0"][:], t_SC[0][:])
                    nc.sync.dma_start(dbg["d_SCl0"][:], t_SCl[0][:])
                    nc.sync.dma_start(dbg["d_KRTh"][:], t_KRTh[:])
                    nc.sync.dma_start(dbg["d_KITh"][:], t_KITh[:])
                    nc.sync.dma_start(dbg["d_KRTl8"][:],
                                      t_KRTl8[:].bitcast(U8))
                    t_dAB = consts.tile([128, 1024], FP32, tag="dAB")
                    nc.vector.tensor_copy(t_dAB, psAB[:])
                    nc.sync.dma_start(dbg["d_AB"][:], t_dAB[:])
                    nc.sync.dma_start(dbg["d_nL"][:], Lt[:])
                    nc.sync.dma_start(dbg["d_sml"][:, 0:1], negmx[:])
                    nc.sync.dma_start(dbg["d_sml"][:, 1:2], rs[:])
                    nc.sync.dma_start(dbg["d_sml"][:, 2:3], rrs[:])
                    nc.sync.dma_start(dbg["d_E"][:], E[:])


            def emit_gh(c):
                # G = E*C (DVE), -H = E*(-S) (Pool); 1/rs folds into og.
                # Emitted one chunk late so the DVE/Pool queues never wait
                # on the exp of the chunk they just fed.
                E = E_of[c]
                Cc = t_SC[c][:, 304:304 + KV]
                nSc = t_SC[c][:, 0:KV]
                if c >= NCHUNK - cfg['drain']:
                    # drain: DVE is idle by now - run G there so the Pool
                    # G->nH serial chain halves for the last chunks
                    nc.vector.tensor_tensor(t_G[c % GB][:, 0:KV], E, Cc, mult)
                else:
                    nc.gpsimd.tensor_tensor(t_G[c % GB][:, 0:KV], E, Cc, mult)
                nc.gpsimd.tensor_tensor(t_nH[c % GB][:, 0:KV], E, nSc, mult)

            def emit_tf(c):
                G = t_G[c % GB]
                nH = t_nH[c % GB]
                pst = ps_ght.tile([128, 768], FP16, tag="ght", name=f"pst{c}")
                for j in range(3):
                    jsl = slice(j * 128, (j + 1) * 128)
                    nc.tensor.transpose(pst[:, j * 128:(j + 1) * 128],
                                        G[:, jsl], t_ident)
                    nc.tensor.transpose(
                        pst[:, 384 + j * 128:384 + (j + 1) * 128],
                        nH[:, jsl], t_ident)
                GHT = ghts.tile([128, 768], FP16, tag="ghts", name=f"GHT{c}")
                dc = 384 if c >= NCHUNK - cfg['drain'] else DC
                if dc > 0:
                    nc.vector.tensor_copy(GHT[:, 0:dc], pst[:, 0:dc])
                if dc < 768:
                    nc.scalar.activation(GHT[:, dc:768], pst[:, dc:768], Copy)
                og = psO[c // 8][:, (c % 8) * 64:(c % 8 + 1) * 64]
                for j in range(3):
                    w = 128 if j < 2 else KV - 256
                    nc.tensor.matmul(og, GHT[0:w, j * 128:(j + 1) * 128],
                                     t_VRsb[0:w, j * 64:(j + 1) * 64],
                                     start=(j == 0), stop=False)
                    nc.tensor.matmul(
                        og, GHT[0:w, 384 + j * 128:384 + (j + 1) * 128],
                        t_VIsb[0:w, j * 64:(j + 1) * 64],
                        start=False, stop=(j == 2))
                # PSUM cannot source a DMA: stage og in SBUF. Plain copy
                # (host divides by rs), batched per 2 chunks at odd c.
                g = c // 8
                if t_og_of[g] is None:
                    t_og_of[g] = work.tile([128, 512], FP32, tag="og",
                                           name=f"og{g}")
                t_og = t_og_of[g]
                if c % 2 == 1:
                    osl = slice((c % 8 - 1) * 64, (c % 8 + 1) * 64)
                    ogs = psO[g][:, osl]
                    if cfg['og_eng'] == 'act' and c < NCHUNK - 2:
                        nc.scalar.activation(t_og[:, osl], ogs, Copy)
                    else:
                        nc.vector.tensor_copy(t_og[:, osl], ogs)
                if cfg['debug'] and c == 0:
                    nc.sync.dma_start(dbg["d_VRsb"][:], t_VRsb[:])
                    nc.sync.dma_start(dbg["d_VIsb"][:], t_VIsb[:])
                q = cfg['og_dma']
                step = 8 // q
                if (c % 8) % step == step - 1:
                    i0 = ((c % 8) // step) * 64 * step
                    i1 = i0 + 64 * step
                    nc.sync.dma_start(out_g[:, g * 512 + i0:g * 512 + i1],
                                      t_og[:, i0:i1])

            vr_emitted = False

            def emit_vr_prep():
                pvr = ps_ght.tile([128, 768], FP16, tag="ght", name="pvr")
                for j in range(3):
                    jsl = slice(j * 128, (j + 1) * 128)
                    nc.tensor.transpose(pvr[:, j * 64:(j + 1) * 64],
                                        t_VRT[:, jsl], t_ident[0:64, 0:64])
                    nc.tensor.transpose(
                        pvr[:, 192 + j * 64:192 + (j + 1) * 64],
                        t_VIT[:, jsl], t_ident[0:64, 0:64])
                nc.vector.tensor_copy(t_VRsb, pvr[:, 0:192])
                nc.vector.tensor_copy(t_VIsb, pvr[:, 192:384])

            SH = cfg['shift']
            psAB_of = [None] * NCHUNK
            for c in range(NCHUNK):
                psAB_of[c] = emit_ab_main(c)
                if c >= 1:
                    emit_ab_corr(c - 1, psAB_of[c - 1])
                if not vr_emitted:
                    emit_vr_prep()
                    vr_emitted = True
                if c >= 1:
                    emit_softmax(c - 1, psAB_of[c - 1])
                if c >= 2:
                    emit_gh(c - 2)
                if c >= 2 + SH:
                    emit_tf(c - 2 - SH)
            emit_ab_corr(NCHUNK - 1, psAB_of[NCHUNK - 1])
            emit_softmax(NCHUNK - 1, psAB_of[NCHUNK - 1])
            emit_gh(NCHUNK - 2)
            emit_gh(NCHUNK - 1)
            for c in range(NCHUNK - 2 - SH, NCHUNK):
                emit_tf(c)
            nc.sync.dma_start(rs_out[:], t_rs[:])

            ps_out.release()
            ps_ght.release()
            ps_ab.release()
    return nc


_NC_CACHE = {}


def _get_nc(cfg=None):
    key = tuple(sorted((cfg or {}).items()))
    if key not in _NC_CACHE:
        _install_bir_patch()
        _NC_CACHE[key] = _build_nc(cfg)
    return _NC_CACHE[key]


# ------------------------------------------------------------- host wrapper
def _kvecs(nk):
    kx = np.arange(0, nk[0] + 1)
    ky = np.arange(-nk[1], nk[1] + 1)
    kz = np.arange(-nk[2], nk[2] + 1)
    KX, KY, KZ = np.meshgrid(kx, ky, kz, indexing="ij")
    return np.stack([KX, KY, KZ], axis=-1).reshape(-1, 3).astype(np.float64)


def _bf16_trunc(x):
    """bf16 truncation (high 16 bits) of fp32, as fp32."""
    xv = np.ascontiguousarray(x, dtype=np.float32).view(np.uint32)
    return (xv & 0xFFFF0000).view(np.float32)


def _prep_core_inputs(q, k, v, r, box):
    f = (r.astype(np.float64) / box[None, :])  # [n,3] in [0,1)

    nk = [max(1, int(b)) for b in (box / DL).astype(np.int64)]
    kvs = _kvecs(nk)
    ksq = TWOPI ** 2 * ((kvs / box[None, :]) ** 2).sum(-1)
    valid = (ksq <= K_SQ_MAX) & (ksq > 0)
    kint = kvs[valid]  # [KV,3] small integers
    assert kint.shape[0] == KV, f"valid k-points {kint.shape[0]} != {KV}"
    kcolT = kint.T  # [3, KV]

    fh = f.astype(np.float16).astype(np.float64)
    fl = (f - fh).astype(np.float16).astype(np.float64)

    lhsb = np.zeros((48, N_NODE), np.float16)
    rhsb = np.zeros((48, KV), np.float16)
    # bank1 (C): accumulate p_h + 1/4 -> round -> -p_h - p_l - 1/4
    lhsb[0:3] = fh.T
    lhsb[3] = 1.0
    lhsb[4] = MAGIC_A
    lhsb[5] = MAGIC_A
    lhsb[6:9] = fh.T
    lhsb[9:12] = fl.T
    lhsb[12] = 1.0
    rhsb[0:3] = kcolT
    rhsb[3] = 0.25
    rhsb[4] = MAGIC_B
    rhsb[5] = -MAGIC_B
    rhsb[6:9] = -kcolT
    rhsb[9:12] = -kcolT
    rhsb[12] = -0.25
    # bank0 (-S): accumulate -p_h -> round -> +p_h + p_l
    lhsb[32:35] = fh.T
    lhsb[35] = MAGIC_A
    lhsb[36] = MAGIC_A
    lhsb[37:40] = fh.T
    lhsb[40:43] = fl.T
    rhsb[32:35] = -kcolT
    rhsb[35] = MAGIC_B
    rhsb[36] = -MAGIC_B
    rhsb[37:40] = kcolT
    rhsb[40:43] = kcolT

    # kv concat: rows 0:64 v, 64:128 k; chunk-major free axis
    kvc = np.concatenate([v, k], axis=1)  # [n,128]
    kvcat = np.ascontiguousarray(
        kvc.reshape(NCHUNK, 128, 128).transpose(1, 0, 2).reshape(128, N_NODE)
    ).astype(np.float32)
    kvl16 = (kvcat - _bf16_trunc(kvcat)).astype(np.float16)

    # q bf16 hi/lo pair, stacked on partitions: rows 0:64 hi, 64:128 lo
    qT = np.ascontiguousarray(q.T).astype(np.float32)  # [64, n]
    qh = qT.astype(ml_dtypes.bfloat16)
    ql = (qT - qh.astype(np.float32)).astype(ml_dtypes.bfloat16)
    qs16 = np.concatenate([qh.view(np.uint16), ql.view(np.uint16)], axis=0)

    # fp8 q/16 with zero slot1 per chunk: [64, c, 2, 128]
    q8 = (qT / 16.0).astype(ml_dtypes.float8_e4m3)
    q8z = np.zeros((64, NCHUNK, 2, 128), ml_dtypes.float8_e4m3)
    q8z[:, :, 0, :] = q8.reshape(64, NCHUNK, 128)

    lhsx = np.zeros((48, N_NODE + 304), np.float16)
    lhsx[:, 0:N_NODE] = lhsb
    lhsx[:, N_NODE:N_NODE + KV] = rhsb
    return {"lhs": lhsx.view(np.uint16),
            "kvcat": kvcat, "kvl": kvl16.view(np.uint16),
            "qs": qs16, "q8z": q8z.reshape(64, -1).view(np.uint8)}


def kernel(q_vector, k_vector, v_vector, positions, cell, batch):
    q_vector = np.asarray(q_vector)
    k_vector = np.asarray(k_vector)
    v_vector = np.asarray(v_vector)
    positions = np.asarray(positions)
    cell = np.asarray(cell)

    n = N_NODE
    boxes = np.diagonal(cell.reshape(-1, 3, 3), axis1=-2, axis2=-1)  # [B,3]

    in_maps = []
    for b in range(B):
        sl = slice(b * n, (b + 1) * n)
        in_maps.append(_prep_core_inputs(
            q_vector[sl], k_vector[sl], v_vector[sl], positions[sl],
            boxes[b].astype(np.float64)))

    nc = _get_nc()
    res = None
    last_err = None
    for _attempt in range(3):
        try:
            res = run_bass_kernel_spmd(nc, in_maps, list(range(B)))
            break
        except Exception as e:  # transient device states
            last_err = e
    if res is None:
        raise last_err

    out = np.empty((B * n, D), np.float32)
    for b in range(B):
        og = res.results[b]["out_g"].reshape(128, 2, 8, D)  # [p, g, c8, d]
        o = og.transpose(1, 2, 0, 3).reshape(n, D)
        rs = res.results[b]["rs_out"].T.reshape(n, 1)  # [c*128+p]
        out[b * n:(b + 1) * n] = o / rs
    return out


if __name__ == "__main__":
    rng = np.random.default_rng(0)
    inputs = {
        "q_vector": rng.standard_normal((B * N_NODE, D), dtype=np.float32),
        "k_vector": rng.standard_normal((B * N_NODE, D), dtype=np.float32),
        "v_vector": rng.standard_normal((B * N_NODE, D), dtype=np.float32),
        "positions": rng.uniform(0, 20, (B * N_NODE, 3)).astype(np.float32),
        "cell": np.tile((np.eye(3, dtype=np.float32) * 20.0)[None], (B, 1, 1)),
        "batch": np.repeat(np.arange(B, dtype=np.int32), N_NODE),
    }
    o = kernel(**inputs)
    print("kernel ran, out", o.shape, o.dtype, float(np.abs(o).max()))
